# revision 1
# baseline (speedup 1.0000x reference)
"""CropAndResize (TF semantics, 32x32 crops, bilinear, extrapolation=0) on 8
Trainium2 NeuronCores via Bass/Tile.

Strategy
--------
ONE SPMD program dispatched once across all 8 cores (a single shard_map'd
PJRT executable). Under the axon tunnel a dispatch costs ~70 ms of round-trip
latency regardless of payload, so 8 per-core dispatches (the old design) pay
8 RTTs when they serialize; one fused dispatch pays exactly one.

Sharding: core k holds channels [32k, 32k+32) of ALL 4 batch images, packed
as a [4*32 = 128 partitions, H*W] slab (20 MiB, SBUF-resident). Box geometry
(sampling rows/cols + lerp weights) derives only from `boxes`/`box_ind`,
which are global — so it is baked into the program as immediates and the
program is identical on every core. Box n computes on partition range
[32*box_ind[n], +32) of the slab; each core emits its 32-channel slice of
all 256 boxes: no redundant compute, no collectives.

Per box (host-precomputed indices/weights): separable bilinear.
  vertical:   vrow[c, i, x] = img[c, t_i, x]*wt_i + img[c, t_i+1, x]*wb_i
              over the box's x-window (split into <=WCAP-wide sub-boxes)
  horizontal: out[c, i, j] = vrow[c, i, l_j]*wl_j + vrow[c, i, l_j+1]*wr_j
Validity masks are folded into the weights. Work is spread across the
Activation, Vector (scalar_tensor_tensor fused MAC) and GPSIMD engines with a
greedy load balancer.
"""

import sys
import threading

sys.path.insert(0, "/opt/trn_rl_repo")

import numpy as np

_B, _C, _H, _W, _CROP = 4, 256, 160, 256, 32
_NCORE = 8
_CPC = _C // _NCORE  # channels per core (32)
_WCAP = 256  # max x-window width of one sub-box (bounds vrow SBUF tile)

# ---------------------------------------------------------------- compat ---
# This container's walrus accepts at most ONE semaphore sync-wait per
# instruction. Patch Tile's kernel-tail drain, and post-rewrite any
# instruction carrying N>1 waits into N-1 preceding single-wait
# EventSemaphore instructions on the same engine.
_ctr = [0]


def _apply_bass_patches():
    import bass_rust
    from concourse.tile import TileContext
    from concourse.vector_clock import ScopedClock

    def _drain_and_barrier_split_waits(self, tick_clock, wait_clock):
        nc = self.nc
        probe = nc.sync.nop()
        wait_clock.add_sem_waits(
            probe.ins, ScopedClock({None: tick_clock.global_clock})
        )
        si = probe.ins.sync_info
        waits = list(si.on_wait) if si is not None else []
        probe.ins.sync_info = None
        name_to_handle = dict(self.sems.allocated().items())
        for w in waits:
            h = name_to_handle.get(w.ant_name)
            if h is not None:
                nc.sync.wait_ge(h, w.wait_value)
            else:
                ev = nc.sync.nop()
                ev.ins.sync_info = bass_rust.SyncInfo(on_wait=[w], on_update=[])
        nc.sync.drain()
        nc.all_engine_barrier()
        popped = nc._tile_sem_poison_stack.pop()
        assert popped is self._sem_poison
        nc.clear_and_free_semaphores(list(self.sems.allocated().values()))
        nc.all_engine_barrier()

    TileContext._drain_and_barrier = _drain_and_barrier_split_waits


def _split_multi_waits(nc):
    import bass_rust
    import concourse.mybir as mybir

    for f in nc.m.functions:
        for bb in f.blocks:
            changed = False
            new = []
            for ins in bb.instructions:
                si = ins.sync_info
                if si is not None and si.on_wait and len(si.on_wait) > 1:
                    changed = True
                    waits = list(si.on_wait)
                    for w in waits[:-1]:
                        _ctr[0] += 1
                        new.append(
                            mybir.InstEventSemaphore(
                                name=f"I-wsplit-{_ctr[0]}",
                                engine=ins.engine,
                                ins=[],
                                outs=[],
                                sync_info=bass_rust.SyncInfo(
                                    on_wait=[w], on_update=[]
                                ),
                            )
                        )
                    ins.sync_info = bass_rust.SyncInfo(
                        on_wait=[waits[-1]], on_update=list(si.on_update)
                    )
                new.append(ins)
            if changed:
                bb.instructions = new


# ------------------------------------------------------------- host plan ---
class _Sub:
    __slots__ = ("xlo", "w", "js", "lL", "lR", "wl", "wr")


class _BoxTask:
    __slots__ = ("n", "B", "t", "b", "wt", "wb", "subs")


def _plan_boxes(boxes, box_ind):
    """Mirror the reference's float32 index math exactly; fold validity into
    the lerp weights. Returns one _BoxTask per box, in box order."""
    f32 = np.float32
    boxes = np.asarray(boxes, dtype=f32)
    n_boxes = boxes.shape[0]
    y1, x1, y2, x2 = boxes[:, 0], boxes[:, 1], boxes[:, 2], boxes[:, 3]
    hs = (y2 - y1) * f32(_H - 1) / f32(_CROP - 1)
    ws = (x2 - x1) * f32(_W - 1) / f32(_CROP - 1)
    ii = np.arange(_CROP, dtype=f32)
    in_y = y1[:, None] * f32(_H - 1) + ii[None, :] * hs[:, None]  # [N, 32]
    in_x = x1[:, None] * f32(_W - 1) + ii[None, :] * ws[:, None]
    vy = (in_y >= 0) & (in_y <= _H - 1)
    vx = (in_x >= 0) & (in_x <= _W - 1)
    top_f = np.floor(in_y)
    left_f = np.floor(in_x)
    ly = (in_y - top_f).astype(f32)
    lx = (in_x - left_f).astype(f32)
    t = np.clip(top_f, 0, _H - 1).astype(np.int64)
    b = np.clip(top_f + 1, 0, _H - 1).astype(np.int64)
    l = np.clip(left_f, 0, _W - 1).astype(np.int64)
    r = np.clip(left_f + 1, 0, _W - 1).astype(np.int64)
    wt = np.where(vy, 1 - ly, 0).astype(f32)
    wb = np.where(vy, ly, 0).astype(f32)
    wl = np.where(vx, 1 - lx, 0).astype(f32)
    wr = np.where(vx, lx, 0).astype(f32)

    tasks = []
    for n in range(n_boxes):
        task = _BoxTask()
        task.n = n
        task.B = int(box_ind[n])
        task.t = t[n]
        task.b = b[n]
        task.wt = wt[n]
        task.wb = wb[n]
        task.subs = []
        j0 = 0
        while j0 < _CROP:
            lo = int(l[n, j0])
            hi = int(r[n, j0])
            j1 = j0 + 1
            while j1 < _CROP:
                nlo = min(lo, int(l[n, j1]))
                nhi = max(hi, int(r[n, j1]))
                if nhi + 1 - nlo > _WCAP:
                    break
                lo, hi = nlo, nhi
                j1 += 1
            sub = _Sub()
            sub.xlo = lo
            sub.w = hi + 1 - lo
            sub.js = list(range(j0, j1))
            sub.lL = [int(l[n, j]) - lo for j in sub.js]
            sub.lR = [int(r[n, j]) - lo for j in sub.js]
            sub.wl = [float(wl[n, j]) for j in sub.js]
            sub.wr = [float(wr[n, j]) for j in sub.js]
            task.subs.append(sub)
            j0 = j1
        tasks.append(task)
    return tasks


# ------------------------------------------------------- device program ---
class _Balancer:
    """Greedy per-MAC-pair engine assignment over ACT / DVE / GPSIMD."""

    def __init__(self):
        self.load = {"ACT": 0.0, "DVE": 0.0, "GPS": 0.0}

    @staticmethod
    def _costs(fd):
        # Engine-busy constants calibrated from an NTFF trace of this very
        # kernel (per-opcode mean durations; see session notes): GPSIMD
        # TENSOR_SCALAR is ~3x the CoreSim estimate, ACT ~351ns, DVE stt
        # ~252ns at fd~100.
        act_mul = 0.9 * fd + 250
        dve_mul = 0.7 * fd + 170  # tensor_scalar (no fast mode observed)
        dve_stt = 1.25 * fd + 170  # scalar_tensor_tensor (no fast mode)
        gps_mul = 2.2 * fd + 550
        gps_add = 2.2 * fd + 300
        return act_mul, dve_mul, dve_stt, gps_mul, gps_add

    def pick(self, fd):
        """Return config name minimizing resulting max-load."""
        act_mul, dve_mul, dve_stt, gps_mul, gps_add = self._costs(fd)
        cfgs = {
            "c1": {"ACT": act_mul, "DVE": dve_stt},  # ACT mul + DVE stt
            "c2": {"GPS": gps_mul, "DVE": dve_stt},  # GPS mul + DVE stt
            "c3": {"DVE": dve_mul + dve_stt},  # all-DVE
            "c4": {"ACT": act_mul, "DVE": dve_mul, "GPS": gps_add},  # 3-op
            "c5": {"ACT": 2 * act_mul, "GPS": gps_add},  # ACT x2 + GPS add
        }
        best, best_m = None, None
        for name, add in cfgs.items():
            m = max(self.load[e] + add.get(e, 0.0) for e in self.load)
            if best_m is None or m < best_m:
                best, best_m = name, m
        for e, v in cfgs[best].items():
            self.load[e] += v
        return best


def _emit_pair(nc, AL, bal, p0, fd, dst, in_a, sa, in_b, sb, tmp_pool, mybir):
    """dst = in_a*sa + in_b*sb via a balanced engine config, on partitions
    [p0, p0+_CPC)."""
    cfg = bal.pick(fd)
    f32 = mybir.dt.float16
    p1 = p0 + _CPC
    if cfg == "c1" or cfg == "c2":
        tmp = tmp_pool.tile([128, fd], f32, tag=f"t{fd <= 32}")
        tv = tmp[p0:p1, :fd]
        if cfg == "c1":
            nc.scalar.mul(out=tv, in_=in_a, mul=sa)
        else:
            nc.gpsimd.tensor_scalar_mul(tv, in_a, sa)
        nc.vector.scalar_tensor_tensor(dst, in_b, sb, tv, AL.mult, AL.add)
    elif cfg == "c3":
        tmp = tmp_pool.tile([128, fd], f32, tag=f"t{fd <= 32}")
        tv = tmp[p0:p1, :fd]
        nc.vector.tensor_scalar_mul(tv, in_a, sa)
        nc.vector.scalar_tensor_tensor(dst, in_b, sb, tv, AL.mult, AL.add)
    else:  # c4 / c5: 3-op, add on GPSIMD
        tmp = tmp_pool.tile([128, fd], f32, tag=f"t{fd <= 32}")
        tmp2 = tmp_pool.tile([128, fd], f32, tag=f"u{fd <= 32}")
        tv = tmp[p0:p1, :fd]
        tv2 = tmp2[p0:p1, :fd]
        nc.scalar.mul(out=tv, in_=in_a, mul=sa)
        if cfg == "c4":
            nc.vector.tensor_scalar_mul(tv2, in_b, sb)
        else:
            nc.scalar.mul(out=tv2, in_=in_b, mul=sb)
        nc.gpsimd.tensor_tensor(dst, tv, tv2, AL.add)


def _build_program(tasks):
    """One program, identical on all cores: each box computes on the
    partition range of its batch inside the [4*32, H*W] channel slab."""
    import concourse.bass as bass
    import concourse.mybir as mybir
    from concourse.tile import TileContext

    AL = mybir.AluOpType
    f32 = mybir.dt.float16
    n_k = len(tasks)
    nc = bass.Bass()
    img_p = nc.declare_dram_parameter("image", [128, _H * _W], f32, isOutput=False)
    out_p = nc.declare_dram_parameter(
        "out", [n_k, _CPC, _CROP * _CROP], f32, isOutput=True
    )
    bal = _Balancer()
    with TileContext(nc) as tc:
        with (
            tc.tile_pool(name="img", bufs=1) as imgp,
            tc.tile_pool(name="vrow", bufs=2) as vrowp,
            tc.tile_pool(name="tmp", bufs=12) as tmpp,
            tc.tile_pool(name="outp", bufs=2) as outp,
        ):
            IMG = imgp.tile([128, _H * _W], f32)
            nc.sync.dma_start(out=IMG[:], in_=img_p[:])
            for k, task in enumerate(tasks):
                p0 = task.B * _CPC
                p1 = p0 + _CPC
                out_t = outp.tile([128, _CROP * _CROP], f32)
                out_v = out_t.rearrange("p (i j) -> p i j", j=_CROP)
                for sub in task.subs:
                    w = sub.w
                    vrow = vrowp.tile([128, _CROP * _WCAP], f32, tag="vrow")
                    for i in range(_CROP):
                        top = IMG[p0:p1, task.t[i] * _W + sub.xlo :][:, :w]
                        bot = IMG[p0:p1, task.b[i] * _W + sub.xlo :][:, :w]
                        dst = vrow[p0:p1, i * w : (i + 1) * w]
                        _emit_pair(
                            nc, AL, bal, p0, w, dst, top, float(task.wt[i]),
                            bot, float(task.wb[i]), tmpp, mybir,
                        )
                    vv = vrow[:, : _CROP * w].rearrange("p (i w) -> p i w", w=w)
                    for jj, j in enumerate(sub.js):
                        VL = vv[p0:p1, :, sub.lL[jj]]
                        VR = vv[p0:p1, :, sub.lR[jj]]
                        dst = out_v[p0:p1, :, j]
                        _emit_pair(
                            nc, AL, bal, p0, _CROP, dst, VL, sub.wl[jj],
                            VR, sub.wr[jj], tmpp, mybir,
                        )
                nc.sync.dma_start(out=out_p[k], in_=out_t[p0:p1, :])
    _split_multi_waits(nc)
    return nc


# ------------------------------------------------------------- execution ---
def _make_spmd_exec(nc, n_cores):
    """Build a persistent shard_map'd jitted callable running the SAME bass
    program on `n_cores` devices (mirrors bass2jax.run_bass_via_pjrt's
    multi-core path, but reusable with device-resident args and no
    donation so repeat runs don't re-upload)."""
    import jax
    import concourse.mybir as mybir
    from concourse import bass2jax
    from jax.experimental.shard_map import shard_map
    from jax.sharding import Mesh, NamedSharding, PartitionSpec

    bass2jax.install_neuronx_cc_hook()

    partition_name = (
        nc.partition_id_tensor.name if nc.partition_id_tensor else None
    )
    in_names, out_names, out_avals, zero_outs = [], [], [], []
    for alloc in nc.m.functions[0].allocations:
        if not isinstance(alloc, mybir.MemoryLocationSet):
            continue
        name = alloc.memorylocations[0].name
        if alloc.kind == "ExternalInput":
            if name != partition_name:
                in_names.append(name)
        elif alloc.kind == "ExternalOutput":
            out_names.append(name)
            shape = tuple(alloc.tensor_shape)
            dtype = mybir.dt.np(alloc.dtype)
            out_avals.append(jax.core.ShapedArray(shape, dtype))
            zero_outs.append(np.zeros(shape, dtype))
    n_params = len(in_names)
    n_outs = len(out_avals)
    all_names = in_names + out_names
    if partition_name is not None:
        all_names = all_names + [partition_name]

    def _body(*args):
        operands = list(args)
        if partition_name is not None:
            operands.append(bass2jax.partition_id_tensor())
        outs = bass2jax._bass_exec_p.bind(
            *operands,
            out_avals=tuple(out_avals),
            in_names=tuple(all_names),
            out_names=tuple(out_names),
            lowering_input_output_aliases=(),
            sim_require_finite=True,
            sim_require_nnan=True,
            nc=nc,
        )
        return tuple(outs)

    devices = jax.devices()[:n_cores]
    assert len(devices) == n_cores, devices
    mesh = Mesh(np.asarray(devices), ("core",))
    sharded = jax.jit(
        shard_map(
            _body,
            mesh=mesh,
            in_specs=(PartitionSpec("core"),) * (n_params + n_outs),
            out_specs=(PartitionSpec("core"),) * len(out_names),
            check_rep=False,
        ),
        keep_unused=True,
    )
    sharding = NamedSharding(mesh, PartitionSpec("core"))
    return sharded, in_names[:n_params], out_names, zero_outs, sharding


class Runner:
    """Compiles the single SPMD program for a given (boxes, box_ind) plan and
    keeps device-resident inputs, so `run()` measures dispatch+execute only."""

    def __init__(self, image, boxes, box_ind):
        import jax

        _apply_bass_patches()
        image = np.ascontiguousarray(np.asarray(image, dtype=np.float32))
        self.boxes = np.asarray(boxes, dtype=np.float32)
        self.n_boxes = self.boxes.shape[0]
        box_ind = np.asarray(box_ind, dtype=np.int32)

        self.tasks = _plan_boxes(self.boxes, box_ind)
        nc = _build_program(self.tasks)
        sharded, in_names, out_names, zero_outs, sharding = _make_spmd_exec(
            nc, _NCORE
        )
        self.sharded = sharded
        self.out_names = out_names

        # Core k's slab: channels [32k, 32k+32) of all batches, partition
        # p = b*32 + (c - 32k).
        slabs = [
            np.ascontiguousarray(
                image[:, k * _CPC : (k + 1) * _CPC]
                .reshape(_B * _CPC, _H * _W)
                .astype(np.float16)
            )
            for k in range(_NCORE)
        ]
        in_map = {"image": np.concatenate(slabs, axis=0)}
        args = [
            jax.device_put(in_map[n], sharding) for n in in_names
        ]
        args += [
            jax.device_put(
                np.zeros((_NCORE * z.shape[0], *z.shape[1:]), z.dtype), sharding
            )
            for z in zero_outs
        ]
        jax.block_until_ready(args)
        self.args = args

    def run(self):
        import jax

        outs = self.sharded(*self.args)
        jax.block_until_ready(outs)
        return outs

    def gather(self, outs):
        n = self.n_boxes
        res = {name: o for name, o in zip(self.out_names, outs)}
        arr = (
            np.asarray(res["out"])
            .astype(np.float32)
            .reshape(_NCORE, n, _CPC, _CROP, _CROP)
        )
        # arr[k, n, cc] is channel k*32+cc of box n.
        out = np.ascontiguousarray(
            arr.transpose(1, 0, 2, 3, 4).reshape(n, _C, _CROP, _CROP)
        )
        return out


def kernel(image, boxes, box_ind):
    r = Runner(image, boxes, box_ind)
    return r.gather(r.run())



# revision 2
# speedup vs baseline: 89.8951x; 89.8951x over previous
"""CropAndResize (TF semantics, 32x32 crops, bilinear, extrapolation=0) on 8
Trainium2 NeuronCores via Bass/Tile.

Strategy
--------
ONE SPMD program dispatched once across all 8 cores (a single shard_map'd
PJRT executable). Under the axon tunnel a dispatch costs ~70 ms of round-trip
latency regardless of payload, so 8 per-core dispatches (the old design) pay
8 RTTs when they serialize; one fused dispatch pays exactly one.

Sharding: core k holds channels [32k, 32k+32) of ALL 4 batch images, packed
as a [4*32 = 128 partitions, H*W] slab (20 MiB, SBUF-resident). Box geometry
(sampling rows/cols + lerp weights) derives only from `boxes`/`box_ind`,
which are global — so it is baked into the program as immediates and the
program is identical on every core. Box n computes on partition range
[32*box_ind[n], +32) of the slab; each core emits its 32-channel slice of
all 256 boxes: no redundant compute, no collectives.

Per box (host-precomputed indices/weights): separable bilinear.
  vertical:   vrow[c, i, x] = img[c, t_i, x]*wt_i + img[c, t_i+1, x]*wb_i
              over the box's x-window (split into <=WCAP-wide sub-boxes)
  horizontal: out[c, i, j] = vrow[c, i, l_j]*wl_j + vrow[c, i, l_j+1]*wr_j
Validity masks are folded into the weights. Work is spread across the
Activation, Vector (scalar_tensor_tensor fused MAC) and GPSIMD engines with a
greedy load balancer.
"""

import sys
import threading

sys.path.insert(0, "/opt/trn_rl_repo")

import numpy as np

_B, _C, _H, _W, _CROP = 4, 256, 160, 256, 32
_NCORE = 8
_CPC = _C // _NCORE  # channels per core (32)
_WCAP = 256  # max x-window width of one sub-box (bounds vrow SBUF tile)

# ---------------------------------------------------------------- compat ---
# This container's walrus accepts at most ONE semaphore sync-wait per
# instruction. Patch Tile's kernel-tail drain, and post-rewrite any
# instruction carrying N>1 waits into N-1 preceding single-wait
# EventSemaphore instructions on the same engine.
_ctr = [0]


def _apply_bass_patches():
    import bass_rust
    from concourse.tile import TileContext
    from concourse.vector_clock import ScopedClock

    def _drain_and_barrier_split_waits(self, tick_clock, wait_clock):
        nc = self.nc
        probe = nc.sync.nop()
        wait_clock.add_sem_waits(
            probe.ins, ScopedClock({None: tick_clock.global_clock})
        )
        si = probe.ins.sync_info
        waits = list(si.on_wait) if si is not None else []
        probe.ins.sync_info = None
        name_to_handle = dict(self.sems.allocated().items())
        for w in waits:
            h = name_to_handle.get(w.ant_name)
            if h is not None:
                nc.sync.wait_ge(h, w.wait_value)
            else:
                ev = nc.sync.nop()
                ev.ins.sync_info = bass_rust.SyncInfo(on_wait=[w], on_update=[])
        nc.sync.drain()
        nc.all_engine_barrier()
        popped = nc._tile_sem_poison_stack.pop()
        assert popped is self._sem_poison
        nc.clear_and_free_semaphores(list(self.sems.allocated().values()))
        nc.all_engine_barrier()

    TileContext._drain_and_barrier = _drain_and_barrier_split_waits


def _split_multi_waits(nc):
    import bass_rust
    import concourse.mybir as mybir

    for f in nc.m.functions:
        for bb in f.blocks:
            changed = False
            new = []
            for ins in bb.instructions:
                si = ins.sync_info
                if si is not None and si.on_wait and len(si.on_wait) > 1:
                    changed = True
                    waits = list(si.on_wait)
                    for w in waits[:-1]:
                        _ctr[0] += 1
                        new.append(
                            mybir.InstEventSemaphore(
                                name=f"I-wsplit-{_ctr[0]}",
                                engine=ins.engine,
                                ins=[],
                                outs=[],
                                sync_info=bass_rust.SyncInfo(
                                    on_wait=[w], on_update=[]
                                ),
                            )
                        )
                    ins.sync_info = bass_rust.SyncInfo(
                        on_wait=[waits[-1]], on_update=list(si.on_update)
                    )
                new.append(ins)
            if changed:
                bb.instructions = new


# ------------------------------------------------------------- host plan ---
class _Sub:
    __slots__ = ("xlo", "w", "js", "lL", "lR", "wl", "wr")


class _BoxTask:
    __slots__ = ("n", "B", "t", "b", "wt", "wb", "subs")


def _plan_boxes(boxes, box_ind):
    """Mirror the reference's float32 index math exactly; fold validity into
    the lerp weights. Returns one _BoxTask per box, in box order."""
    f32 = np.float32
    boxes = np.asarray(boxes, dtype=f32)
    n_boxes = boxes.shape[0]
    y1, x1, y2, x2 = boxes[:, 0], boxes[:, 1], boxes[:, 2], boxes[:, 3]
    hs = (y2 - y1) * f32(_H - 1) / f32(_CROP - 1)
    ws = (x2 - x1) * f32(_W - 1) / f32(_CROP - 1)
    ii = np.arange(_CROP, dtype=f32)
    in_y = y1[:, None] * f32(_H - 1) + ii[None, :] * hs[:, None]  # [N, 32]
    in_x = x1[:, None] * f32(_W - 1) + ii[None, :] * ws[:, None]
    vy = (in_y >= 0) & (in_y <= _H - 1)
    vx = (in_x >= 0) & (in_x <= _W - 1)
    top_f = np.floor(in_y)
    left_f = np.floor(in_x)
    ly = (in_y - top_f).astype(f32)
    lx = (in_x - left_f).astype(f32)
    t = np.clip(top_f, 0, _H - 1).astype(np.int64)
    b = np.clip(top_f + 1, 0, _H - 1).astype(np.int64)
    l = np.clip(left_f, 0, _W - 1).astype(np.int64)
    r = np.clip(left_f + 1, 0, _W - 1).astype(np.int64)
    wt = np.where(vy, 1 - ly, 0).astype(f32)
    wb = np.where(vy, ly, 0).astype(f32)
    wl = np.where(vx, 1 - lx, 0).astype(f32)
    wr = np.where(vx, lx, 0).astype(f32)

    tasks = []
    for n in range(n_boxes):
        task = _BoxTask()
        task.n = n
        task.B = int(box_ind[n])
        task.t = t[n]
        task.b = b[n]
        task.wt = wt[n]
        task.wb = wb[n]
        task.subs = []
        j0 = 0
        while j0 < _CROP:
            lo = int(l[n, j0])
            hi = int(r[n, j0])
            j1 = j0 + 1
            while j1 < _CROP:
                nlo = min(lo, int(l[n, j1]))
                nhi = max(hi, int(r[n, j1]))
                if nhi + 1 - nlo > _WCAP:
                    break
                lo, hi = nlo, nhi
                j1 += 1
            sub = _Sub()
            sub.xlo = lo
            sub.w = hi + 1 - lo
            sub.js = list(range(j0, j1))
            sub.lL = [int(l[n, j]) - lo for j in sub.js]
            sub.lR = [int(r[n, j]) - lo for j in sub.js]
            sub.wl = [float(wl[n, j]) for j in sub.js]
            sub.wr = [float(wr[n, j]) for j in sub.js]
            task.subs.append(sub)
            j0 = j1
        tasks.append(task)
    return tasks


# ------------------------------------------------------- device program ---
class _Balancer:
    """Greedy per-MAC-pair engine assignment over ACT / DVE / GPSIMD."""

    def __init__(self):
        self.load = {"ACT": 0.0, "DVE": 0.0, "GPS": 0.0}

    @staticmethod
    def _costs(fd):
        # Engine-busy constants calibrated from an NTFF trace of this very
        # kernel (per-opcode mean durations; see session notes): GPSIMD
        # TENSOR_SCALAR is ~3x the CoreSim estimate, ACT ~351ns, DVE stt
        # ~252ns at fd~100.
        act_mul = 0.9 * fd + 250
        dve_mul = 0.7 * fd + 170  # tensor_scalar (no fast mode observed)
        dve_stt = 1.25 * fd + 170  # scalar_tensor_tensor (no fast mode)
        gps_mul = 2.2 * fd + 550
        gps_add = 2.2 * fd + 300
        return act_mul, dve_mul, dve_stt, gps_mul, gps_add

    def pick(self, fd):
        """Return config name minimizing resulting max-load."""
        act_mul, dve_mul, dve_stt, gps_mul, gps_add = self._costs(fd)
        cfgs = {
            "c1": {"ACT": act_mul, "DVE": dve_stt},  # ACT mul + DVE stt
            "c2": {"GPS": gps_mul, "DVE": dve_stt},  # GPS mul + DVE stt
            "c3": {"DVE": dve_mul + dve_stt},  # all-DVE
            "c4": {"ACT": act_mul, "DVE": dve_mul, "GPS": gps_add},  # 3-op
            "c5": {"ACT": 2 * act_mul, "GPS": gps_add},  # ACT x2 + GPS add
        }
        best, best_m = None, None
        for name, add in cfgs.items():
            m = max(self.load[e] + add.get(e, 0.0) for e in self.load)
            if best_m is None or m < best_m:
                best, best_m = name, m
        for e, v in cfgs[best].items():
            self.load[e] += v
        return best


def _emit_pair(nc, AL, bal, p0, fd, dst, in_a, sa, in_b, sb, tmp_pool, mybir):
    """dst = in_a*sa + in_b*sb via a balanced engine config, on partitions
    [p0, p0+_CPC)."""
    cfg = bal.pick(fd)
    f32 = mybir.dt.float16
    p1 = p0 + _CPC
    if cfg == "c1" or cfg == "c2":
        tmp = tmp_pool.tile([128, fd], f32, tag=f"t{fd <= 32}")
        tv = tmp[p0:p1, :fd]
        if cfg == "c1":
            nc.scalar.mul(out=tv, in_=in_a, mul=sa)
        else:
            nc.gpsimd.tensor_scalar_mul(tv, in_a, sa)
        nc.vector.scalar_tensor_tensor(dst, in_b, sb, tv, AL.mult, AL.add)
    elif cfg == "c3":
        tmp = tmp_pool.tile([128, fd], f32, tag=f"t{fd <= 32}")
        tv = tmp[p0:p1, :fd]
        nc.vector.tensor_scalar_mul(tv, in_a, sa)
        nc.vector.scalar_tensor_tensor(dst, in_b, sb, tv, AL.mult, AL.add)
    else:  # c4 / c5: 3-op, add on GPSIMD
        tmp = tmp_pool.tile([128, fd], f32, tag=f"t{fd <= 32}")
        tmp2 = tmp_pool.tile([128, fd], f32, tag=f"u{fd <= 32}")
        tv = tmp[p0:p1, :fd]
        tv2 = tmp2[p0:p1, :fd]
        nc.scalar.mul(out=tv, in_=in_a, mul=sa)
        if cfg == "c4":
            nc.vector.tensor_scalar_mul(tv2, in_b, sb)
        else:
            nc.scalar.mul(out=tv2, in_=in_b, mul=sb)
        nc.gpsimd.tensor_tensor(dst, tv, tv2, AL.add)


def _build_program(tasks):
    """One program, identical on all cores: each box computes on the
    partition range of its batch inside the [4*32, H*W] channel slab."""
    import concourse.bass as bass
    import concourse.mybir as mybir
    from concourse.tile import TileContext

    AL = mybir.AluOpType
    f32 = mybir.dt.float16
    n_k = len(tasks)
    nc = bass.Bass()
    img_p = nc.declare_dram_parameter("image", [128, _H * _W], f32, isOutput=False)
    out_p = nc.declare_dram_parameter(
        "out", [n_k, _CPC, _CROP * _CROP], f32, isOutput=True
    )
    bal = _Balancer()
    with TileContext(nc) as tc:
        with (
            tc.tile_pool(name="img", bufs=1) as imgp,
            tc.tile_pool(name="vrow", bufs=2) as vrowp,
            tc.tile_pool(name="tmp", bufs=12) as tmpp,
            tc.tile_pool(name="outp", bufs=2) as outp,
        ):
            IMG = imgp.tile([128, _H * _W], f32)
            nc.sync.dma_start(out=IMG[:], in_=img_p[:])
            for k, task in enumerate(tasks):
                p0 = task.B * _CPC
                p1 = p0 + _CPC
                out_t = outp.tile([128, _CROP * _CROP], f32)
                out_v = out_t.rearrange("p (i j) -> p i j", j=_CROP)
                for sub in task.subs:
                    w = sub.w
                    vrow = vrowp.tile([128, _CROP * _WCAP], f32, tag="vrow")
                    for i in range(_CROP):
                        top = IMG[p0:p1, task.t[i] * _W + sub.xlo :][:, :w]
                        bot = IMG[p0:p1, task.b[i] * _W + sub.xlo :][:, :w]
                        dst = vrow[p0:p1, i * w : (i + 1) * w]
                        _emit_pair(
                            nc, AL, bal, p0, w, dst, top, float(task.wt[i]),
                            bot, float(task.wb[i]), tmpp, mybir,
                        )
                    vv = vrow[:, : _CROP * w].rearrange("p (i w) -> p i w", w=w)
                    for jj, j in enumerate(sub.js):
                        VL = vv[p0:p1, :, sub.lL[jj]]
                        VR = vv[p0:p1, :, sub.lR[jj]]
                        dst = out_v[p0:p1, :, j]
                        _emit_pair(
                            nc, AL, bal, p0, _CROP, dst, VL, sub.wl[jj],
                            VR, sub.wr[jj], tmpp, mybir,
                        )
                nc.sync.dma_start(out=out_p[k], in_=out_t[p0:p1, :])
    _split_multi_waits(nc)
    return nc


# ------------------------------------------------------------- execution ---
def _make_spmd_exec(nc, n_cores):
    """Build a persistent shard_map'd jitted callable running the SAME bass
    program on `n_cores` devices (mirrors bass2jax.run_bass_via_pjrt's
    multi-core path, but reusable with device-resident args and no
    donation so repeat runs don't re-upload)."""
    import jax
    import concourse.mybir as mybir
    from concourse import bass2jax
    from jax.experimental.shard_map import shard_map
    from jax.sharding import Mesh, NamedSharding, PartitionSpec

    bass2jax.install_neuronx_cc_hook()

    partition_name = (
        nc.partition_id_tensor.name if nc.partition_id_tensor else None
    )
    in_names, out_names, out_avals, zero_outs = [], [], [], []
    for alloc in nc.m.functions[0].allocations:
        if not isinstance(alloc, mybir.MemoryLocationSet):
            continue
        name = alloc.memorylocations[0].name
        if alloc.kind == "ExternalInput":
            if name != partition_name:
                in_names.append(name)
        elif alloc.kind == "ExternalOutput":
            out_names.append(name)
            shape = tuple(alloc.tensor_shape)
            dtype = mybir.dt.np(alloc.dtype)
            out_avals.append(jax.core.ShapedArray(shape, dtype))
            zero_outs.append(np.zeros(shape, dtype))
    n_params = len(in_names)
    n_outs = len(out_avals)
    all_names = in_names + out_names
    if partition_name is not None:
        all_names = all_names + [partition_name]

    def _body(*args):
        operands = list(args)
        if partition_name is not None:
            operands.append(bass2jax.partition_id_tensor())
        outs = bass2jax._bass_exec_p.bind(
            *operands,
            out_avals=tuple(out_avals),
            in_names=tuple(all_names),
            out_names=tuple(out_names),
            lowering_input_output_aliases=(),
            sim_require_finite=True,
            sim_require_nnan=True,
            nc=nc,
        )
        return tuple(outs)

    devices = jax.devices()[:n_cores]
    assert len(devices) == n_cores, devices
    mesh = Mesh(np.asarray(devices), ("core",))
    sharded = jax.jit(
        shard_map(
            _body,
            mesh=mesh,
            in_specs=(PartitionSpec("core"),) * (n_params + n_outs),
            out_specs=(PartitionSpec("core"),) * len(out_names),
            check_rep=False,
        ),
        keep_unused=True,
    )
    sharding = NamedSharding(mesh, PartitionSpec("core"))
    return sharded, in_names[:n_params], out_names, zero_outs, sharding


class Runner:
    """Compiles the single SPMD program for a given (boxes, box_ind) plan and
    keeps device-resident inputs, so `run()` measures dispatch+execute only."""

    def __init__(self, image, boxes, box_ind):
        import jax

        _apply_bass_patches()
        image = np.ascontiguousarray(np.asarray(image, dtype=np.float32))
        self.boxes = np.asarray(boxes, dtype=np.float32)
        self.n_boxes = self.boxes.shape[0]
        box_ind = np.asarray(box_ind, dtype=np.int32)

        self.tasks = _plan_boxes(self.boxes, box_ind)
        nc = _build_program(self.tasks)
        self.nc = nc
        sharded, in_names, out_names, zero_outs, sharding = _make_spmd_exec(
            nc, _NCORE
        )
        self.sharded = sharded
        self.out_names = out_names

        # Core k's slab: channels [32k, 32k+32) of all batches, partition
        # p = b*32 + (c - 32k).
        slabs = [
            np.ascontiguousarray(
                image[:, k * _CPC : (k + 1) * _CPC]
                .reshape(_B * _CPC, _H * _W)
                .astype(np.float16)
            )
            for k in range(_NCORE)
        ]
        in_map = {"image": np.concatenate(slabs, axis=0)}
        args = [
            jax.device_put(in_map[n], sharding) for n in in_names
        ]
        args += [
            jax.device_put(
                np.zeros((_NCORE * z.shape[0], *z.shape[1:]), z.dtype), sharding
            )
            for z in zero_outs
        ]
        jax.block_until_ready(args)
        self.args = args

    def run(self):
        import jax

        outs = self.sharded(*self.args)
        jax.block_until_ready(outs)
        return outs

    def gather(self, outs):
        n = self.n_boxes
        res = {name: o for name, o in zip(self.out_names, outs)}
        arr = (
            np.asarray(res["out"])
            .astype(np.float32)
            .reshape(_NCORE, n, _CPC, _CROP, _CROP)
        )
        # arr[k, n, cc] is channel k*32+cc of box n.
        out = np.ascontiguousarray(
            arr.transpose(1, 0, 2, 3, 4).reshape(n, _C, _CROP, _CROP)
        )
        return out


def kernel(image, boxes, box_ind):
    r = Runner(image, boxes, box_ind)
    return r.gather(r.run())



# revision 3
# speedup vs baseline: 101.7624x; 1.1320x over previous
"""CropAndResize v2: PE-matmul vertical pass + lane-parallel horizontal pass.

Sharding: core k owns channels [32k, 32k+32) of all 4 batch images (per the
shard-over-N/channel-replica hint family; geometry is global so one SPMD
program runs on all 8 cores).

Per-core layout: image slab with Y ON PARTITIONS and c INNERMOST (so the
horizontal gather reads contiguous 32-channel runs):
  slabA[p = y,      b*8192 + x*32 + c]  for y in 0..127
  slabB[p = y - 32, b*8192 + x*32 + c]  for y in 32..159  (overlap copy)
A box's y-window [y0, y1] (y1-y0 < 128 by construction of segments) lives in
slabA if y1 <= 127, slabB if y0 >= 32, else split at the 127/128 boundary.

Per box n (all host-precomputed geometry, baked as immediates):
  1. Horizontal tap products, lane-parallel over y partitions (one lane per
     image row in the window): for each constant-stride run of l_j
     (Bresenham runs of the sample grid):
       prod[y, t*1024 + j*32 + c] = slab[y, b0, l_j+t, c] * WLR[n, j, t]
     (DVE/GPS; reads stream contiguous 32-channel runs).
  2. Vertical lerp AND the tap-pair fold as PE matmuls: the two prod
     halves accumulate into one psum:
       psum[i, (j,c)] = V_n^T @ prod[:, t=0 half] + V_n^T @ prod[:, t=1 half]
     where V_n [K, 32] holds wt_i at row t_i and wb_i at row t_i+1 (2
     nonzeros per column, dense f16 stationary, zero rows annihilate the
     out-of-window garbage lanes). All moving operands are contiguous.
  3. ACT copies psum (f32) -> f16 staging; DMA to DRAM out[n].

Weight tables (WLR horizontal pairs, V_packed vertical columns) are built on
host from `boxes` exactly mirroring the reference f32 index math and uploaded
once; they are identical on every core.
"""

import sys

sys.path.insert(0, "/opt/trn_rl_repo")

import numpy as np

_B, _C, _H, _W, _CROP = 4, 256, 160, 256, 32
_NCORE = 8
_CPC = _C // _NCORE  # 32

# ---------------------------------------------------------------- compat ---
# This container's walrus accepts at most ONE semaphore sync-wait per
# instruction. Patch Tile's kernel-tail drain, and post-rewrite any
# instruction carrying N>1 waits into N-1 preceding single-wait
# EventSemaphore instructions on the same engine.
_ctr = [0]


def _apply_bass_patches():
    import bass_rust
    from concourse.tile import TileContext
    from concourse.vector_clock import ScopedClock

    def _drain_and_barrier_split_waits(self, tick_clock, wait_clock):
        nc = self.nc
        probe = nc.sync.nop()
        wait_clock.add_sem_waits(
            probe.ins, ScopedClock({None: tick_clock.global_clock})
        )
        si = probe.ins.sync_info
        waits = list(si.on_wait) if si is not None else []
        probe.ins.sync_info = None
        name_to_handle = dict(self.sems.allocated().items())
        for w in waits:
            h = name_to_handle.get(w.ant_name)
            if h is not None:
                nc.sync.wait_ge(h, w.wait_value)
            else:
                ev = nc.sync.nop()
                ev.ins.sync_info = bass_rust.SyncInfo(on_wait=[w], on_update=[])
        nc.sync.drain()
        nc.all_engine_barrier()
        popped = nc._tile_sem_poison_stack.pop()
        assert popped is self._sem_poison
        nc.clear_and_free_semaphores(list(self.sems.allocated().values()))
        nc.all_engine_barrier()

    TileContext._drain_and_barrier = _drain_and_barrier_split_waits


def _split_multi_waits(nc):
    import bass_rust
    import concourse.mybir as mybir

    for f in nc.m.functions:
        for bb in f.blocks:
            changed = False
            new = []
            for ins in bb.instructions:
                si = ins.sync_info
                if si is not None and si.on_wait and len(si.on_wait) > 1:
                    changed = True
                    waits = list(si.on_wait)
                    for w in waits[:-1]:
                        _ctr[0] += 1
                        new.append(
                            mybir.InstEventSemaphore(
                                name=f"I-wsplit-{_ctr[0]}",
                                engine=ins.engine,
                                ins=[],
                                outs=[],
                                sync_info=bass_rust.SyncInfo(
                                    on_wait=[w], on_update=[]
                                ),
                            )
                        )
                    ins.sync_info = bass_rust.SyncInfo(
                        on_wait=[waits[-1]], on_update=list(si.on_update)
                    )
                new.append(ins)
            if changed:
                bb.instructions = new


# ------------------------------------------------------------- host plan ---
class _Box:
    __slots__ = ("n", "b0", "segs", "runs", "rank")
    # segs: list of (slab: 'A'|'B', pbase, K, vblk)
    #   lanes are ABSOLUTE slab partitions [pbase, pbase+K); rows outside the
    #   box's y-window ride along as garbage and are annihilated by zero rows
    #   of the V block (partition ranges must start at 0/32/64/96 and not
    #   cross their alignment boundary, so we can't window at y0).
    # runs: generalized 2-level runs (j0, p, alen, l0, P, wl0, wr0):
    #   covers j = j0 + p*a for a < alen with l_j = l0 + P*a. The Bresenham
    #   sequence l_j is quasi-periodic, so decimating by its period p makes
    #   the subsequences affine — far fewer ops than 1st-difference runs.


def _gen_runs(ln):
    """Minimal cover of j=0..31 by 2-level affine runs of l, over period p."""
    best = None
    for p in range(1, 9):
        ops = []
        for r in range(p):
            js = list(range(r, _CROP, p))
            a = 0
            while a < len(js):
                b = a + 1
                P = None
                while b < len(js):
                    nP = int(ln[js[b]] - ln[js[b - 1]])
                    if P is None or nP == P:
                        P = nP
                        b += 1
                    else:
                        break
                ops.append(
                    (js[a], p, b - a, int(ln[js[a]]), P if P is not None else 0)
                )
                a = b
        if best is None or len(ops) < len(best):
            best = ops
    return best


def _plan(boxes, box_ind, n_emit=None):
    """Mirror the reference's float32 index math exactly. Returns
    (box plans, WLR table [128,16384] f16, V_packed [128, 32*nblk] f16)."""
    f32 = np.float32
    boxes = np.asarray(boxes, dtype=f32)
    N = boxes.shape[0]
    y1c, x1c, y2c, x2c = boxes[:, 0], boxes[:, 1], boxes[:, 2], boxes[:, 3]
    hs = (y2c - y1c) * f32(_H - 1) / f32(_CROP - 1)
    ws = (x2c - x1c) * f32(_W - 1) / f32(_CROP - 1)
    ii = np.arange(_CROP, dtype=f32)
    in_y = y1c[:, None] * f32(_H - 1) + ii[None, :] * hs[:, None]  # [N, 32]
    in_x = x1c[:, None] * f32(_W - 1) + ii[None, :] * ws[:, None]
    vy = (in_y >= 0) & (in_y <= _H - 1)
    vx = (in_x >= 0) & (in_x <= _W - 1)
    top_f = np.floor(in_y)
    left_f = np.floor(in_x)
    ly = (in_y - top_f).astype(f32)
    lx = (in_x - left_f).astype(f32)
    t = np.clip(top_f, 0, _H - 1).astype(np.int64)
    l = np.clip(left_f, 0, _W - 1).astype(np.int64)
    wt = np.where(vy, 1 - ly, 0).astype(f32)
    wb = np.where(vy, ly, 0).astype(f32)
    wl = np.where(vx, 1 - lx, 0).astype(f32)
    wr = np.where(vx, lx, 0).astype(f32)
    # Make the tap pair physically (l, l+1) / (t, t+1): at the top clip edge
    # the reference collapses to a single tap; shift down one with weight 0.
    ml = l == _W - 1
    wr = np.where(ml, wl + wr, wr)
    wl = np.where(ml, 0, wl)
    l = np.where(ml, _W - 2, l)
    mt = t == _H - 1
    wb = np.where(mt, wt + wb, wb)
    wt = np.where(mt, 0, wt)
    t = np.where(mt, _H - 2, t)

    n_emit = N if n_emit is None else n_emit

    plans = []
    for n in range(n_emit):
        bx = _Box()
        bx.n = n
        bx.b0 = int(box_ind[n])
        y0 = int(t[n].min())
        y1 = int(t[n].max()) + 1
        if y1 <= 127:
            # slabA, lanes p = y in [0, y1]
            seg_bounds = [("A", 0, y1 + 1, 0, y1, 0)]
        elif y0 >= 32:
            # slabB, lanes p = y-32 in [0, y1-32]
            seg_bounds = [("B", 0, y1 - 31, y0, y1, 32)]
        else:
            # split at 127/128: slabA rows y0..127 (p=y), slabB rows
            # 128..y1 (p=y-32 in [96, y1-32])
            seg_bounds = [
                ("A", 0, 128, 0, 127, 0),
                ("B", 96, y1 - 127, 128, y1, 32),
            ]
        bx.segs = seg_bounds  # vblk assigned after sorting
        bx.runs = [
            (
                j0,
                p,
                alen,
                l0,
                P,
                tuple(
                    (float(wl[n, j0 + p * a]), float(wr[n, j0 + p * a]))
                    for a in range(alen if alen <= 2 else 0)
                ),
            )
            for (j0, p, alen, l0, P) in _gen_runs(l[n])
        ]
        plans.append(bx)

    # emission order: pure-A boxes (grouped by batch, tracking slab-chunk
    # arrival) first, then B/split boxes (slabB is staged last)
    plans.sort(
        key=lambda bx: (
            0 if (bx.segs[0][0] == "A" and len(bx.segs) == 1) else 1,
            bx.b0,
            bx.n,
        )
    )

    # tables indexed by emission rank so chunked table DMAs gate only
    # later-ranked boxes
    wlr = np.zeros((n_emit * 64,), np.float32)
    vcols = []
    for rank, bx in enumerate(plans):
        bx.rank = rank
        wlr[rank * 64 + 0 : rank * 64 + 64 : 2] = wl[bx.n]
        wlr[rank * 64 + 1 : rank * 64 + 64 : 2] = wr[bx.n]
        segs = []
        for slab, pbase, K, ylo, yhi, pshift in bx.segs:
            blk = np.zeros((128, 32), np.float32)
            for i in range(_CROP):
                ti = int(t[bx.n, i])
                if ylo <= ti <= yhi:
                    blk[ti - pshift, i] += wt[bx.n, i]
                if ylo <= ti + 1 <= yhi:
                    blk[ti + 1 - pshift, i] += wb[bx.n, i]
            segs.append((slab, pbase, K, len(vcols)))
            vcols.append(blk)
        bx.segs = segs

    wlr_tile = np.broadcast_to(
        wlr.astype(np.float16), (128, n_emit * 64)
    ).copy()
    vpk = np.concatenate(vcols, axis=1).astype(np.float16)  # [128, 32*nblk]
    return plans, wlr_tile, vpk


# ------------------------------------------------------- device program ---
class _Bal:
    """Greedy DVE/GPS/ACT load balancer (costs ns, fit from NTFF traces)."""

    def __init__(self):
        self.load = {"V": 0.0, "G": 0.0, "A": 0.0}

    def pick(self, fd, act_ok=False, act_cost=None):
        cands = {"V": 1.45 * fd + 230.0, "G": 2.2 * fd + 440.0}
        if act_ok:
            cands["A"] = act_cost
        best = min(cands, key=lambda e: self.load[e] + cands[e])
        self.load[best] += cands[best]
        return best

    def charge(self, eng, ns):
        self.load[eng] += ns


def _build_program(plans, nblk):
    import concourse.bass as bass
    import concourse.mybir as mybir
    from concourse.ap import AP
    from concourse.tile import TileContext

    AL = mybir.AluOpType
    f16 = mybir.dt.float16
    fp32 = mybir.dt.float32
    n_emit = len(plans)
    nc = bass.Bass()
    imgA_p = nc.declare_dram_parameter("imgA", [128, 32768], f16, isOutput=False)
    imgC_p = nc.declare_dram_parameter("imgC", [32, 32768], f16, isOutput=False)
    wlr_p = nc.declare_dram_parameter(
        "wlr", [128, n_emit * 64], f16, isOutput=False
    )
    vpk_p = nc.declare_dram_parameter("vpk", [128, 32 * nblk], f16, isOutput=False)
    out_p = nc.declare_dram_parameter(
        "out", [n_emit, 32, 1024], f16, isOutput=True
    )
    bal = _Bal()
    with TileContext(nc) as tc:
        with (
            tc.tile_pool(name="img", bufs=1) as imgp,
            tc.tile_pool(name="prod", bufs=4) as prodp,
            tc.tile_pool(name="psum", bufs=4, space="PSUM") as psump,
            tc.tile_pool(name="outp", bufs=6) as outp,
        ):
            SLA = imgp.tile([128, 32768], f16)
            SLB = imgp.tile([128, 32768], f16)
            WLR = imgp.tile([128, n_emit * 64], f16)
            VPK = imgp.tile([128, 32 * nblk], f16)
            # Staging is interleaved into the box loop (emitted just before
            # the first box of each batch group) so early boxes' output DMAs
            # aren't queued behind all 28MB of staging on the DMA engines.
            # plans are sorted (pure-A by batch, then B/split), and tables
            # are rank-indexed, so each group's table columns are a
            # contiguous range.
            groups = {}
            for r, bx in enumerate(plans):
                gk = (
                    0 if (bx.segs[0][0] == "A" and len(bx.segs) == 1) else 1,
                    bx.b0 if (bx.segs[0][0] == "A" and len(bx.segs) == 1) else -1,
                )
                lo_r, hi_r, lo_v, hi_v = groups.get(
                    gk, (r, r, bx.segs[0][3], bx.segs[-1][3])
                )
                groups[gk] = (
                    min(lo_r, r),
                    max(hi_r, r),
                    min(lo_v, bx.segs[0][3]),
                    max(hi_v, bx.segs[-1][3]),
                )

            staged = set()

            def stage_group(gk):
                if gk in staged or gk not in groups:
                    return
                staged.add(gk)
                lo_r, hi_r, lo_v, hi_v = groups[gk]
                ws = slice(lo_r * 64, (hi_r + 1) * 64)
                vs = slice(lo_v * 32, (hi_v + 1) * 32)
                nc.sync.dma_start(out=WLR[:, ws], in_=wlr_p[:, ws])
                nc.sync.dma_start(out=VPK[:, vs], in_=vpk_p[:, vs])
                if gk[0] == 0:
                    cs = slice(gk[1] * 8192, (gk[1] + 1) * 8192)
                    nc.sync.dma_start(out=SLA[:, cs], in_=imgA_p[:, cs])
                    if gk[1] == _B - 1:
                        # slabB fill once all slabA chunks are staged; B/split
                        # boxes run at the end, well after this drains
                        for b in range(_B):
                            cb = slice(b * 8192, (b + 1) * 8192)
                            nc.sync.dma_start(
                                out=SLB[0:96, cb], in_=SLA[32:128, cb]
                            )
                            nc.sync.dma_start(
                                out=SLB[96:128, cb], in_=imgC_p[:, cb]
                            )
                else:
                    # B/split group: ensure every slab chunk is staged
                    for b in range(_B):
                        stage_group((0, b))
            h_sla = SLA[0:1, 0:1].tensor
            h_slb = SLB[0:1, 0:1].tensor
            h_wlr = WLR[0:1, 0:1].tensor

            for bx in plans:
                n = bx.n
                rank = bx.rank
                b0 = bx.b0
                stage_group(
                    (
                        0
                        if (bx.segs[0][0] == "A" and len(bx.segs) == 1)
                        else 1,
                        b0
                        if (bx.segs[0][0] == "A" and len(bx.segs) == 1)
                        else -1,
                    )
                )
                psum = psump.tile([32, 1024], fp32)
                nseg = len(bx.segs)
                for s, (slab, pbase, K, vblk) in enumerate(bx.segs):
                    h_sl = h_sla if slab == "A" else h_slb
                    prod = prodp.tile([128, 2048], f16, tag="prod")
                    h_prod = prod[0:1, 0:1].tensor
                    wlr_fs = n_emit * 64
                    for (j0, p, alen, l0, P, ws) in bx.runs:
                        fd = 64 * alen
                        eng = bal.pick(
                            fd,
                            act_ok=(alen <= 2),
                            act_cost=alen * 2 * 310.0,
                        )
                        if eng == "A":
                            # tiny run: immediate-scalar muls on ACT
                            for a, (wl0, wr0) in enumerate(ws):
                                sbase = (
                                    pbase * 32768
                                    + b0 * 8192
                                    + (l0 + P * a) * 32
                                )
                                obase = pbase * 2048 + (j0 + p * a) * 32
                                for tt, w in ((0, wl0), (1, wr0)):
                                    i0 = AP(
                                        h_sl,
                                        sbase + 32 * tt,
                                        [[32768, K], [1, 32]],
                                    )
                                    ot = AP(
                                        h_prod,
                                        obase + 1024 * tt,
                                        [[2048, K], [1, 32]],
                                    )
                                    nc.scalar.mul(out=ot, in_=i0, mul=w)
                            continue
                        in0 = AP(
                            h_sl,
                            pbase * 32768 + b0 * 8192 + l0 * 32,
                            [[32768, K], [32 * P, alen], [32, 2], [1, 32]],
                        )
                        in1 = AP(
                            h_wlr,
                            pbase * wlr_fs + rank * 64 + j0 * 2,
                            [[wlr_fs, K], [2 * p, alen], [1, 2], [0, 32]],
                        )
                        o = AP(
                            h_prod,
                            pbase * 2048 + j0 * 32,
                            [[2048, K], [32 * p, alen], [1024, 2], [1, 32]],
                        )
                        if eng == "V":
                            nc.vector.tensor_tensor(o, in0, in1, AL.mult)
                        else:
                            nc.gpsimd.tensor_tensor(o, in0, in1, AL.mult)
                    # vertical lerp + tap fold on PE: psum[i, (j,c)] =
                    # sum_t V_seg^T @ prod[:, t-half]; all rhs contiguous.
                    tp = (96, 0) if pbase == 96 else None
                    lhsT = VPK[pbase : pbase + K, vblk * 32 : (vblk + 1) * 32]
                    for t in (0, 1):
                        for h in (0, 1):
                            rhs = prod[
                                pbase : pbase + K,
                                t * 1024 + h * 512 : t * 1024 + (h + 1) * 512,
                            ]
                            nc.tensor.matmul(
                                psum[:, h * 512 : (h + 1) * 512],
                                lhsT,
                                rhs,
                                start=(s == 0 and t == 0),
                                stop=(s == nseg - 1 and t == 1),
                                tile_position=tp,
                            )
                outst = outp.tile([32, 1024], f16, tag="outst")
                nc.scalar.mul(out=outst[:, :], in_=psum[:, :], mul=1.0)
                bal.charge("A", 1110.0)
                nc.sync.dma_start(out=out_p[n], in_=outst[:, :])
    _split_multi_waits(nc)
    return nc


# ------------------------------------------------------------- execution ---
def _make_spmd_exec(nc, n_cores):
    """Persistent shard_map'd jitted callable running the SAME bass program
    on n_cores devices with device-resident args."""
    import jax
    import concourse.mybir as mybir
    from concourse import bass2jax
    from jax.experimental.shard_map import shard_map
    from jax.sharding import Mesh, NamedSharding, PartitionSpec

    bass2jax.install_neuronx_cc_hook()

    partition_name = (
        nc.partition_id_tensor.name if nc.partition_id_tensor else None
    )
    in_names, out_names, out_avals, zero_outs = [], [], [], []
    for alloc in nc.m.functions[0].allocations:
        if not isinstance(alloc, mybir.MemoryLocationSet):
            continue
        name = alloc.memorylocations[0].name
        if alloc.kind == "ExternalInput":
            if name != partition_name:
                in_names.append(name)
        elif alloc.kind == "ExternalOutput":
            out_names.append(name)
            shape = tuple(alloc.tensor_shape)
            dtype = mybir.dt.np(alloc.dtype)
            out_avals.append(jax.core.ShapedArray(shape, dtype))
            zero_outs.append(np.zeros(shape, dtype))
    n_params = len(in_names)
    n_outs = len(out_avals)
    all_names = in_names + out_names
    if partition_name is not None:
        all_names = all_names + [partition_name]

    def _body(*args):
        operands = list(args)
        if partition_name is not None:
            operands.append(bass2jax.partition_id_tensor())
        outs = bass2jax._bass_exec_p.bind(
            *operands,
            out_avals=tuple(out_avals),
            in_names=tuple(all_names),
            out_names=tuple(out_names),
            lowering_input_output_aliases=(),
            sim_require_finite=True,
            sim_require_nnan=True,
            nc=nc,
        )
        return tuple(outs)

    devices = jax.devices()[:n_cores]
    assert len(devices) == n_cores, devices
    mesh = Mesh(np.asarray(devices), ("core",))
    sharded = jax.jit(
        shard_map(
            _body,
            mesh=mesh,
            in_specs=(PartitionSpec("core"),) * (n_params + n_outs),
            out_specs=(PartitionSpec("core"),) * len(out_names),
            check_rep=False,
        ),
        keep_unused=True,
    )
    sharding = NamedSharding(mesh, PartitionSpec("core"))
    return sharded, in_names[:n_params], out_names, zero_outs, sharding


class Runner:
    def __init__(self, image, boxes, box_ind, n_emit=None):
        import jax

        _apply_bass_patches()
        image = np.ascontiguousarray(np.asarray(image, dtype=np.float32))
        boxes = np.asarray(boxes, dtype=np.float32)
        box_ind = np.asarray(box_ind, dtype=np.int32)
        self.n_boxes = boxes.shape[0]

        plans, wlr_tile, vpk = _plan(boxes, box_ind, n_emit)
        self.n_emit = len(plans)
        nblk = vpk.shape[1] // 32
        nc = _build_program(plans, nblk)
        self.nc = nc
        sharded, in_names, out_names, zero_outs, sharding = _make_spmd_exec(
            nc, _NCORE
        )
        self.sharded = sharded
        self.out_names = out_names

        # per-core slabs: core k holds channels [32k, 32k+32).
        img16 = image.astype(np.float16)  # [B, C, H, W]
        # slab col layout: b*8192 + x*32 + c  (c innermost)
        imgA = np.stack(
            [
                img16[:, k * _CPC : (k + 1) * _CPC, 0:128]
                .transpose(2, 0, 3, 1)
                .reshape(128, 32768)
                for k in range(_NCORE)
            ]
        )  # [8, 128, 32768]
        imgC = np.stack(
            [
                img16[:, k * _CPC : (k + 1) * _CPC, 128:160]
                .transpose(2, 0, 3, 1)
                .reshape(32, 32768)
                for k in range(_NCORE)
            ]
        )
        in_map = {
            "imgA": imgA.reshape(_NCORE * 128, 32768),
            "imgC": imgC.reshape(_NCORE * 32, 32768),
            "wlr": np.tile(wlr_tile, (_NCORE, 1)),
            "vpk": np.tile(vpk, (_NCORE, 1)),
        }
        args = [jax.device_put(in_map[n], sharding) for n in in_names]
        args += [
            jax.device_put(
                np.zeros((_NCORE * z.shape[0], *z.shape[1:]), z.dtype), sharding
            )
            for z in zero_outs
        ]
        jax.block_until_ready(args)
        self.args = args

    def run(self):
        import jax

        outs = self.sharded(*self.args)
        jax.block_until_ready(outs)
        return outs

    def gather(self, outs):
        ne = self.n_emit
        res = {name: o for name, o in zip(self.out_names, outs)}
        arr = (
            np.asarray(res["out"])
            .astype(np.float32)
            .reshape(_NCORE, ne, 32, 32, 32)  # [k, n, i, j, c]
        )
        out = np.ascontiguousarray(
            arr.transpose(1, 0, 4, 2, 3).reshape(ne, _C, _CROP, _CROP)
        )
        return out


def kernel(image, boxes, box_ind):
    r = Runner(image, boxes, box_ind)
    return r.gather(r.run())


# revision 4
# speedup vs baseline: 102.5816x; 1.0080x over previous
"""CropAndResize v2: PE-matmul vertical pass + lane-parallel horizontal pass.

Sharding: core k owns channels [32k, 32k+32) of all 4 batch images (per the
shard-over-N/channel-replica hint family; geometry is global so one SPMD
program runs on all 8 cores).

Per-core layout: image slab with Y ON PARTITIONS and c INNERMOST (so the
horizontal gather reads contiguous 32-channel runs):
  slabA[p = y,      b*8192 + x*32 + c]  for y in 0..127
  slabB[p = y - 32, b*8192 + x*32 + c]  for y in 32..159  (overlap copy)
A box's y-window [y0, y1] (y1-y0 < 128 by construction of segments) lives in
slabA if y1 <= 127, slabB if y0 >= 32, else split at the 127/128 boundary.

Per box n (all host-precomputed geometry, baked as immediates):
  1. Horizontal tap products, lane-parallel over y partitions (one lane per
     image row in the window): for each constant-stride run of l_j
     (Bresenham runs of the sample grid):
       prod[y, t*1024 + j*32 + c] = slab[y, b0, l_j+t, c] * WLR[n, j, t]
     (DVE/GPS; reads stream contiguous 32-channel runs).
  2. Vertical lerp AND the tap-pair fold as PE matmuls: the two prod
     halves accumulate into one psum:
       psum[i, (j,c)] = V_n^T @ prod[:, t=0 half] + V_n^T @ prod[:, t=1 half]
     where V_n [K, 32] holds wt_i at row t_i and wb_i at row t_i+1 (2
     nonzeros per column, dense f16 stationary, zero rows annihilate the
     out-of-window garbage lanes). All moving operands are contiguous.
  3. ACT copies psum (f32) -> f16 staging; DMA to DRAM out[n].

Weight tables (WLR horizontal pairs, V_packed vertical columns) are built on
host from `boxes` exactly mirroring the reference f32 index math and uploaded
once; they are identical on every core.
"""

import sys

sys.path.insert(0, "/opt/trn_rl_repo")

import numpy as np

_B, _C, _H, _W, _CROP = 4, 256, 160, 256, 32
_NCORE = 8
_CPC = _C // _NCORE  # 32

# ---------------------------------------------------------------- compat ---
# This container's walrus accepts at most ONE semaphore sync-wait per
# instruction. Patch Tile's kernel-tail drain, and post-rewrite any
# instruction carrying N>1 waits into N-1 preceding single-wait
# EventSemaphore instructions on the same engine.
_ctr = [0]


def _apply_bass_patches():
    import bass_rust
    from concourse.tile import TileContext
    from concourse.vector_clock import ScopedClock

    def _drain_and_barrier_split_waits(self, tick_clock, wait_clock):
        nc = self.nc
        probe = nc.sync.nop()
        wait_clock.add_sem_waits(
            probe.ins, ScopedClock({None: tick_clock.global_clock})
        )
        si = probe.ins.sync_info
        waits = list(si.on_wait) if si is not None else []
        probe.ins.sync_info = None
        name_to_handle = dict(self.sems.allocated().items())
        for w in waits:
            h = name_to_handle.get(w.ant_name)
            if h is not None:
                nc.sync.wait_ge(h, w.wait_value)
            else:
                ev = nc.sync.nop()
                ev.ins.sync_info = bass_rust.SyncInfo(on_wait=[w], on_update=[])
        nc.sync.drain()
        nc.all_engine_barrier()
        popped = nc._tile_sem_poison_stack.pop()
        assert popped is self._sem_poison
        nc.clear_and_free_semaphores(list(self.sems.allocated().values()))
        nc.all_engine_barrier()

    TileContext._drain_and_barrier = _drain_and_barrier_split_waits


def _split_multi_waits(nc):
    import bass_rust
    import concourse.mybir as mybir

    for f in nc.m.functions:
        for bb in f.blocks:
            changed = False
            new = []
            for ins in bb.instructions:
                si = ins.sync_info
                if si is not None and si.on_wait and len(si.on_wait) > 1:
                    changed = True
                    waits = list(si.on_wait)
                    for w in waits[:-1]:
                        _ctr[0] += 1
                        new.append(
                            mybir.InstEventSemaphore(
                                name=f"I-wsplit-{_ctr[0]}",
                                engine=ins.engine,
                                ins=[],
                                outs=[],
                                sync_info=bass_rust.SyncInfo(
                                    on_wait=[w], on_update=[]
                                ),
                            )
                        )
                    ins.sync_info = bass_rust.SyncInfo(
                        on_wait=[waits[-1]], on_update=list(si.on_update)
                    )
                new.append(ins)
            if changed:
                bb.instructions = new


# ------------------------------------------------------------- host plan ---
class _Box:
    __slots__ = ("n", "b0", "segs", "runs", "rank")
    # segs: list of (slab: 'A'|'B', pbase, K, vblk)
    #   lanes are ABSOLUTE slab partitions [pbase, pbase+K); rows outside the
    #   box's y-window ride along as garbage and are annihilated by zero rows
    #   of the V block (partition ranges must start at 0/32/64/96 and not
    #   cross their alignment boundary, so we can't window at y0).
    # runs: generalized 2-level runs (j0, p, alen, l0, P, wl0, wr0):
    #   covers j = j0 + p*a for a < alen with l_j = l0 + P*a. The Bresenham
    #   sequence l_j is quasi-periodic, so decimating by its period p makes
    #   the subsequences affine — far fewer ops than 1st-difference runs.


def _gen_runs(ln):
    """Minimal cover of j=0..31 by 2-level affine runs of l, over period p."""
    best = None
    for p in range(1, 9):
        ops = []
        for r in range(p):
            js = list(range(r, _CROP, p))
            a = 0
            while a < len(js):
                b = a + 1
                P = None
                while b < len(js):
                    nP = int(ln[js[b]] - ln[js[b - 1]])
                    if P is None or nP == P:
                        P = nP
                        b += 1
                    else:
                        break
                ops.append(
                    (js[a], p, b - a, int(ln[js[a]]), P if P is not None else 0)
                )
                a = b
        if best is None or len(ops) < len(best):
            best = ops
    return best


def _plan(boxes, box_ind, n_emit=None):
    """Mirror the reference's float32 index math exactly. Returns
    (box plans, WLR table [128,16384] f16, V_packed [128, 32*nblk] f16)."""
    f32 = np.float32
    boxes = np.asarray(boxes, dtype=f32)
    N = boxes.shape[0]
    y1c, x1c, y2c, x2c = boxes[:, 0], boxes[:, 1], boxes[:, 2], boxes[:, 3]
    hs = (y2c - y1c) * f32(_H - 1) / f32(_CROP - 1)
    ws = (x2c - x1c) * f32(_W - 1) / f32(_CROP - 1)
    ii = np.arange(_CROP, dtype=f32)
    in_y = y1c[:, None] * f32(_H - 1) + ii[None, :] * hs[:, None]  # [N, 32]
    in_x = x1c[:, None] * f32(_W - 1) + ii[None, :] * ws[:, None]
    vy = (in_y >= 0) & (in_y <= _H - 1)
    vx = (in_x >= 0) & (in_x <= _W - 1)
    top_f = np.floor(in_y)
    left_f = np.floor(in_x)
    ly = (in_y - top_f).astype(f32)
    lx = (in_x - left_f).astype(f32)
    t = np.clip(top_f, 0, _H - 1).astype(np.int64)
    l = np.clip(left_f, 0, _W - 1).astype(np.int64)
    wt = np.where(vy, 1 - ly, 0).astype(f32)
    wb = np.where(vy, ly, 0).astype(f32)
    wl = np.where(vx, 1 - lx, 0).astype(f32)
    wr = np.where(vx, lx, 0).astype(f32)
    # Make the tap pair physically (l, l+1) / (t, t+1): at the top clip edge
    # the reference collapses to a single tap; shift down one with weight 0.
    ml = l == _W - 1
    wr = np.where(ml, wl + wr, wr)
    wl = np.where(ml, 0, wl)
    l = np.where(ml, _W - 2, l)
    mt = t == _H - 1
    wb = np.where(mt, wt + wb, wb)
    wt = np.where(mt, 0, wt)
    t = np.where(mt, _H - 2, t)

    n_emit = N if n_emit is None else n_emit

    plans = []
    for n in range(n_emit):
        bx = _Box()
        bx.n = n
        bx.b0 = int(box_ind[n])
        y0 = int(t[n].min())
        y1 = int(t[n].max()) + 1
        if y1 <= 127:
            # slabA, lanes p = y in [0, y1]
            seg_bounds = [("A", 0, y1 + 1, 0, y1, 0)]
        elif y0 >= 32:
            # slabB, lanes p = y-32 in [0, y1-32]
            seg_bounds = [("B", 0, y1 - 31, y0, y1, 32)]
        else:
            # split at 127/128: slabA rows y0..127 (p=y), slabB rows
            # 128..y1 (p=y-32 in [96, y1-32])
            seg_bounds = [
                ("A", 0, 128, 0, 127, 0),
                ("B", 96, y1 - 127, 128, y1, 32),
            ]
        bx.segs = seg_bounds  # vblk assigned after sorting
        bx.runs = [
            (
                j0,
                p,
                alen,
                l0,
                P,
                tuple(
                    (float(wl[n, j0 + p * a]), float(wr[n, j0 + p * a]))
                    for a in range(alen if alen <= 4 else 0)
                ),
            )
            for (j0, p, alen, l0, P) in _gen_runs(l[n])
        ]
        plans.append(bx)

    # emission order: pure-A boxes (grouped by batch, tracking slab-chunk
    # arrival) first, then B/split boxes (slabB is staged last)
    plans.sort(
        key=lambda bx: (
            0 if (bx.segs[0][0] == "A" and len(bx.segs) == 1) else 1,
            bx.b0,
            bx.n,
        )
    )

    # tables indexed by emission rank so chunked table DMAs gate only
    # later-ranked boxes
    wlr = np.zeros((n_emit * 64,), np.float32)
    vcols = []
    for rank, bx in enumerate(plans):
        bx.rank = rank
        wlr[rank * 64 + 0 : rank * 64 + 64 : 2] = wl[bx.n]
        wlr[rank * 64 + 1 : rank * 64 + 64 : 2] = wr[bx.n]
        segs = []
        for slab, pbase, K, ylo, yhi, pshift in bx.segs:
            blk = np.zeros((128, 32), np.float32)
            for i in range(_CROP):
                ti = int(t[bx.n, i])
                if ylo <= ti <= yhi:
                    blk[ti - pshift, i] += wt[bx.n, i]
                if ylo <= ti + 1 <= yhi:
                    blk[ti + 1 - pshift, i] += wb[bx.n, i]
            segs.append((slab, pbase, K, len(vcols)))
            vcols.append(blk)
        bx.segs = segs

    wlr_tile = np.broadcast_to(
        wlr.astype(np.float16), (128, n_emit * 64)
    ).copy()
    vpk = np.concatenate(vcols, axis=1).astype(np.float16)  # [128, 32*nblk]
    return plans, wlr_tile, vpk


# ------------------------------------------------------- device program ---
class _Bal:
    """Greedy DVE/GPS/ACT load balancer (costs ns, fit from NTFF traces)."""

    def __init__(self):
        self.load = {"V": 0.0, "G": 0.0, "A": 0.0}

    def pick(self, fd, act_ok=False, act_cost=None):
        cands = {"V": 1.55 * fd + 230.0, "G": 2.15 * fd + 440.0}
        if act_ok:
            cands["A"] = act_cost
        best = min(cands, key=lambda e: self.load[e] + cands[e])
        self.load[best] += cands[best]
        return best

    def charge(self, eng, ns):
        self.load[eng] += ns


def _build_program(plans, nblk):
    import concourse.bass as bass
    import concourse.mybir as mybir
    from concourse.ap import AP
    from concourse.tile import TileContext

    AL = mybir.AluOpType
    f16 = mybir.dt.float16
    fp32 = mybir.dt.float32
    n_emit = len(plans)
    nc = bass.Bass()
    imgA_p = nc.declare_dram_parameter("imgA", [128, 32768], f16, isOutput=False)
    imgC_p = nc.declare_dram_parameter("imgC", [32, 32768], f16, isOutput=False)
    wlr_p = nc.declare_dram_parameter(
        "wlr", [128, n_emit * 64], f16, isOutput=False
    )
    vpk_p = nc.declare_dram_parameter("vpk", [128, 32 * nblk], f16, isOutput=False)
    out_p = nc.declare_dram_parameter(
        "out", [n_emit, 32, 1024], f16, isOutput=True
    )
    bal = _Bal()
    with TileContext(nc) as tc:
        with (
            tc.tile_pool(name="img", bufs=1) as imgp,
            tc.tile_pool(name="prod", bufs=4) as prodp,
            tc.tile_pool(name="psum", bufs=4, space="PSUM") as psump,
            tc.tile_pool(name="outp", bufs=6) as outp,
        ):
            SLA = imgp.tile([128, 32768], f16)
            SLB = imgp.tile([128, 32768], f16)
            WLR = imgp.tile([128, n_emit * 64], f16)
            VPK = imgp.tile([128, 32 * nblk], f16)
            # Staging is interleaved into the box loop (emitted just before
            # the first box of each batch group) so early boxes' output DMAs
            # aren't queued behind all 28MB of staging on the DMA engines.
            # plans are sorted (pure-A by batch, then B/split), and tables
            # are rank-indexed, so each group's table columns are a
            # contiguous range.
            groups = {}
            for r, bx in enumerate(plans):
                gk = (
                    0 if (bx.segs[0][0] == "A" and len(bx.segs) == 1) else 1,
                    bx.b0 if (bx.segs[0][0] == "A" and len(bx.segs) == 1) else -1,
                )
                lo_r, hi_r, lo_v, hi_v = groups.get(
                    gk, (r, r, bx.segs[0][3], bx.segs[-1][3])
                )
                groups[gk] = (
                    min(lo_r, r),
                    max(hi_r, r),
                    min(lo_v, bx.segs[0][3]),
                    max(hi_v, bx.segs[-1][3]),
                )

            staged = set()

            def stage_group(gk):
                if gk in staged or gk not in groups:
                    return
                staged.add(gk)
                lo_r, hi_r, lo_v, hi_v = groups[gk]
                ws = slice(lo_r * 64, (hi_r + 1) * 64)
                vs = slice(lo_v * 32, (hi_v + 1) * 32)
                nc.sync.dma_start(out=WLR[:, ws], in_=wlr_p[:, ws])
                nc.sync.dma_start(out=VPK[:, vs], in_=vpk_p[:, vs])
                if gk[0] == 0:
                    cs = slice(gk[1] * 8192, (gk[1] + 1) * 8192)
                    nc.sync.dma_start(out=SLA[:, cs], in_=imgA_p[:, cs])
                    if gk[1] == _B - 1:
                        # slabB fill once all slabA chunks are staged; B/split
                        # boxes run at the end, well after this drains
                        for b in range(_B):
                            cb = slice(b * 8192, (b + 1) * 8192)
                            nc.sync.dma_start(
                                out=SLB[0:96, cb], in_=SLA[32:128, cb]
                            )
                            nc.sync.dma_start(
                                out=SLB[96:128, cb], in_=imgC_p[:, cb]
                            )
                else:
                    # B/split group: ensure every slab chunk is staged
                    for b in range(_B):
                        stage_group((0, b))
            h_sla = SLA[0:1, 0:1].tensor
            h_slb = SLB[0:1, 0:1].tensor
            h_wlr = WLR[0:1, 0:1].tensor

            for bx in plans:
                n = bx.n
                rank = bx.rank
                b0 = bx.b0
                stage_group(
                    (
                        0
                        if (bx.segs[0][0] == "A" and len(bx.segs) == 1)
                        else 1,
                        b0
                        if (bx.segs[0][0] == "A" and len(bx.segs) == 1)
                        else -1,
                    )
                )
                psum = psump.tile([32, 1024], fp32)
                nseg = len(bx.segs)
                for s, (slab, pbase, K, vblk) in enumerate(bx.segs):
                    h_sl = h_sla if slab == "A" else h_slb
                    prod = prodp.tile([128, 2048], f16, tag="prod")
                    h_prod = prod[0:1, 0:1].tensor
                    wlr_fs = n_emit * 64
                    for (j0, p, alen, l0, P, ws) in bx.runs:
                        fd = 64 * alen
                        eng = bal.pick(
                            fd,
                            act_ok=(alen <= 4),
                            act_cost=alen * 2 * 310.0,
                        )
                        if eng == "A":
                            # tiny run: immediate-scalar muls on ACT
                            for a, (wl0, wr0) in enumerate(ws):
                                sbase = (
                                    pbase * 32768
                                    + b0 * 8192
                                    + (l0 + P * a) * 32
                                )
                                obase = pbase * 2048 + (j0 + p * a) * 32
                                for tt, w in ((0, wl0), (1, wr0)):
                                    i0 = AP(
                                        h_sl,
                                        sbase + 32 * tt,
                                        [[32768, K], [1, 32]],
                                    )
                                    ot = AP(
                                        h_prod,
                                        obase + 1024 * tt,
                                        [[2048, K], [1, 32]],
                                    )
                                    nc.scalar.mul(out=ot, in_=i0, mul=w)
                            continue
                        in0 = AP(
                            h_sl,
                            pbase * 32768 + b0 * 8192 + l0 * 32,
                            [[32768, K], [32 * P, alen], [32, 2], [1, 32]],
                        )
                        in1 = AP(
                            h_wlr,
                            pbase * wlr_fs + rank * 64 + j0 * 2,
                            [[wlr_fs, K], [2 * p, alen], [1, 2], [0, 32]],
                        )
                        o = AP(
                            h_prod,
                            pbase * 2048 + j0 * 32,
                            [[2048, K], [32 * p, alen], [1024, 2], [1, 32]],
                        )
                        if eng == "V":
                            nc.vector.tensor_tensor(o, in0, in1, AL.mult)
                        else:
                            nc.gpsimd.tensor_tensor(o, in0, in1, AL.mult)
                    # vertical lerp + tap fold on PE: psum[i, (j,c)] =
                    # sum_t V_seg^T @ prod[:, t-half]; all rhs contiguous.
                    tp = (96, 0) if pbase == 96 else None
                    lhsT = VPK[pbase : pbase + K, vblk * 32 : (vblk + 1) * 32]
                    for t in (0, 1):
                        for h in (0, 1):
                            rhs = prod[
                                pbase : pbase + K,
                                t * 1024 + h * 512 : t * 1024 + (h + 1) * 512,
                            ]
                            nc.tensor.matmul(
                                psum[:, h * 512 : (h + 1) * 512],
                                lhsT,
                                rhs,
                                start=(s == 0 and t == 0),
                                stop=(s == nseg - 1 and t == 1),
                                tile_position=tp,
                            )
                outst = outp.tile([32, 1024], f16, tag="outst")
                nc.scalar.mul(out=outst[:, :], in_=psum[:, :], mul=1.0)
                bal.charge("A", 1110.0)
                nc.sync.dma_start(out=out_p[n], in_=outst[:, :])
    _split_multi_waits(nc)
    return nc


# ------------------------------------------------------------- execution ---
def _make_spmd_exec(nc, n_cores):
    """Persistent shard_map'd jitted callable running the SAME bass program
    on n_cores devices with device-resident args."""
    import jax
    import concourse.mybir as mybir
    from concourse import bass2jax
    from jax.experimental.shard_map import shard_map
    from jax.sharding import Mesh, NamedSharding, PartitionSpec

    bass2jax.install_neuronx_cc_hook()

    partition_name = (
        nc.partition_id_tensor.name if nc.partition_id_tensor else None
    )
    in_names, out_names, out_avals, zero_outs = [], [], [], []
    for alloc in nc.m.functions[0].allocations:
        if not isinstance(alloc, mybir.MemoryLocationSet):
            continue
        name = alloc.memorylocations[0].name
        if alloc.kind == "ExternalInput":
            if name != partition_name:
                in_names.append(name)
        elif alloc.kind == "ExternalOutput":
            out_names.append(name)
            shape = tuple(alloc.tensor_shape)
            dtype = mybir.dt.np(alloc.dtype)
            out_avals.append(jax.core.ShapedArray(shape, dtype))
            zero_outs.append(np.zeros(shape, dtype))
    n_params = len(in_names)
    n_outs = len(out_avals)
    all_names = in_names + out_names
    if partition_name is not None:
        all_names = all_names + [partition_name]

    def _body(*args):
        operands = list(args)
        if partition_name is not None:
            operands.append(bass2jax.partition_id_tensor())
        outs = bass2jax._bass_exec_p.bind(
            *operands,
            out_avals=tuple(out_avals),
            in_names=tuple(all_names),
            out_names=tuple(out_names),
            lowering_input_output_aliases=(),
            sim_require_finite=True,
            sim_require_nnan=True,
            nc=nc,
        )
        return tuple(outs)

    devices = jax.devices()[:n_cores]
    assert len(devices) == n_cores, devices
    mesh = Mesh(np.asarray(devices), ("core",))
    sharded = jax.jit(
        shard_map(
            _body,
            mesh=mesh,
            in_specs=(PartitionSpec("core"),) * (n_params + n_outs),
            out_specs=(PartitionSpec("core"),) * len(out_names),
            check_rep=False,
        ),
        keep_unused=True,
    )
    sharding = NamedSharding(mesh, PartitionSpec("core"))
    return sharded, in_names[:n_params], out_names, zero_outs, sharding


class Runner:
    def __init__(self, image, boxes, box_ind, n_emit=None):
        import jax

        _apply_bass_patches()
        image = np.ascontiguousarray(np.asarray(image, dtype=np.float32))
        boxes = np.asarray(boxes, dtype=np.float32)
        box_ind = np.asarray(box_ind, dtype=np.int32)
        self.n_boxes = boxes.shape[0]

        plans, wlr_tile, vpk = _plan(boxes, box_ind, n_emit)
        self.n_emit = len(plans)
        nblk = vpk.shape[1] // 32
        nc = _build_program(plans, nblk)
        self.nc = nc
        sharded, in_names, out_names, zero_outs, sharding = _make_spmd_exec(
            nc, _NCORE
        )
        self.sharded = sharded
        self.out_names = out_names

        # per-core slabs: core k holds channels [32k, 32k+32).
        img16 = image.astype(np.float16)  # [B, C, H, W]
        # slab col layout: b*8192 + x*32 + c  (c innermost)
        imgA = np.stack(
            [
                img16[:, k * _CPC : (k + 1) * _CPC, 0:128]
                .transpose(2, 0, 3, 1)
                .reshape(128, 32768)
                for k in range(_NCORE)
            ]
        )  # [8, 128, 32768]
        imgC = np.stack(
            [
                img16[:, k * _CPC : (k + 1) * _CPC, 128:160]
                .transpose(2, 0, 3, 1)
                .reshape(32, 32768)
                for k in range(_NCORE)
            ]
        )
        in_map = {
            "imgA": imgA.reshape(_NCORE * 128, 32768),
            "imgC": imgC.reshape(_NCORE * 32, 32768),
            "wlr": np.tile(wlr_tile, (_NCORE, 1)),
            "vpk": np.tile(vpk, (_NCORE, 1)),
        }
        args = [jax.device_put(in_map[n], sharding) for n in in_names]
        args += [
            jax.device_put(
                np.zeros((_NCORE * z.shape[0], *z.shape[1:]), z.dtype), sharding
            )
            for z in zero_outs
        ]
        jax.block_until_ready(args)
        self.args = args

    def run(self):
        import jax

        outs = self.sharded(*self.args)
        jax.block_until_ready(outs)
        return outs

    def gather(self, outs):
        ne = self.n_emit
        res = {name: o for name, o in zip(self.out_names, outs)}
        arr = (
            np.asarray(res["out"])
            .astype(np.float32)
            .reshape(_NCORE, ne, 32, 32, 32)  # [k, n, i, j, c]
        )
        out = np.ascontiguousarray(
            arr.transpose(1, 0, 4, 2, 3).reshape(ne, _C, _CROP, _CROP)
        )
        return out


def kernel(image, boxes, box_ind):
    r = Runner(image, boxes, box_ind)
    return r.gather(r.run())


# revision 5
# speedup vs baseline: 106.7643x; 1.0408x over previous
"""CropAndResize v2: PE-matmul vertical pass + lane-parallel horizontal pass.

Sharding: core k owns channels [32k, 32k+32) of all 4 batch images (per the
shard-over-N/channel-replica hint family; geometry is global so one SPMD
program runs on all 8 cores).

Per-core layout: image slab with Y ON PARTITIONS and c INNERMOST (so the
horizontal gather reads contiguous 32-channel runs):
  slabA[p = y,      b*8192 + x*32 + c]  for y in 0..127
  slabB[p = y - 32, b*8192 + x*32 + c]  for y in 32..159  (overlap copy)
A box's y-window [y0, y1] (y1-y0 < 128 by construction of segments) lives in
slabA if y1 <= 127, slabB if y0 >= 32, else split at the 127/128 boundary.

Per box n (all host-precomputed geometry, baked as immediates):
  1. Horizontal tap products, lane-parallel over y partitions (one lane per
     image row in the window): for each constant-stride run of l_j
     (Bresenham runs of the sample grid):
       prod[y, t*1024 + j*32 + c] = slab[y, b0, l_j+t, c] * WLR[n, j, t]
     (DVE/GPS; reads stream contiguous 32-channel runs).
  2. Vertical lerp AND the tap-pair fold as PE matmuls: the two prod
     halves accumulate into one psum:
       psum[i, (j,c)] = V_n^T @ prod[:, t=0 half] + V_n^T @ prod[:, t=1 half]
     where V_n [K, 32] holds wt_i at row t_i and wb_i at row t_i+1 (2
     nonzeros per column, dense f16 stationary, zero rows annihilate the
     out-of-window garbage lanes). All moving operands are contiguous.
  3. ACT copies psum (f32) -> f16 staging; DMA to DRAM out[n].

Weight tables (WLR horizontal pairs, V_packed vertical columns) are built on
host from `boxes` exactly mirroring the reference f32 index math and uploaded
once; they are identical on every core.
"""

import sys

sys.path.insert(0, "/opt/trn_rl_repo")

import numpy as np

_B, _C, _H, _W, _CROP = 4, 256, 160, 256, 32
_NCORE = 8
_CPC = _C // _NCORE  # 32

# ---------------------------------------------------------------- compat ---
# This container's walrus accepts at most ONE semaphore sync-wait per
# instruction. Patch Tile's kernel-tail drain, and post-rewrite any
# instruction carrying N>1 waits into N-1 preceding single-wait
# EventSemaphore instructions on the same engine.
_ctr = [0]


def _apply_bass_patches():
    import bass_rust
    from concourse.tile import TileContext
    from concourse.vector_clock import ScopedClock

    def _drain_and_barrier_split_waits(self, tick_clock, wait_clock):
        nc = self.nc
        probe = nc.sync.nop()
        wait_clock.add_sem_waits(
            probe.ins, ScopedClock({None: tick_clock.global_clock})
        )
        si = probe.ins.sync_info
        waits = list(si.on_wait) if si is not None else []
        probe.ins.sync_info = None
        name_to_handle = dict(self.sems.allocated().items())
        for w in waits:
            h = name_to_handle.get(w.ant_name)
            if h is not None:
                nc.sync.wait_ge(h, w.wait_value)
            else:
                ev = nc.sync.nop()
                ev.ins.sync_info = bass_rust.SyncInfo(on_wait=[w], on_update=[])
        nc.sync.drain()
        nc.all_engine_barrier()
        popped = nc._tile_sem_poison_stack.pop()
        assert popped is self._sem_poison
        nc.clear_and_free_semaphores(list(self.sems.allocated().values()))
        nc.all_engine_barrier()

    TileContext._drain_and_barrier = _drain_and_barrier_split_waits


def _split_multi_waits(nc):
    import bass_rust
    import concourse.mybir as mybir

    for f in nc.m.functions:
        for bb in f.blocks:
            changed = False
            new = []
            for ins in bb.instructions:
                si = ins.sync_info
                if si is not None and si.on_wait and len(si.on_wait) > 1:
                    changed = True
                    waits = list(si.on_wait)
                    for w in waits[:-1]:
                        _ctr[0] += 1
                        new.append(
                            mybir.InstEventSemaphore(
                                name=f"I-wsplit-{_ctr[0]}",
                                engine=ins.engine,
                                ins=[],
                                outs=[],
                                sync_info=bass_rust.SyncInfo(
                                    on_wait=[w], on_update=[]
                                ),
                            )
                        )
                    ins.sync_info = bass_rust.SyncInfo(
                        on_wait=[waits[-1]], on_update=list(si.on_update)
                    )
                new.append(ins)
            if changed:
                bb.instructions = new


# ------------------------------------------------------------- host plan ---
class _Box:
    __slots__ = ("n", "b0", "segs", "runs", "rank")
    # segs: list of (slab: 'A'|'B', pbase, K, vblk)
    #   lanes are ABSOLUTE slab partitions [pbase, pbase+K); rows outside the
    #   box's y-window ride along as garbage and are annihilated by zero rows
    #   of the V block (partition ranges must start at 0/32/64/96 and not
    #   cross their alignment boundary, so we can't window at y0).
    # runs: generalized 2-level runs (j0, p, alen, l0, P, wl0, wr0):
    #   covers j = j0 + p*a for a < alen with l_j = l0 + P*a. The Bresenham
    #   sequence l_j is quasi-periodic, so decimating by its period p makes
    #   the subsequences affine — far fewer ops than 1st-difference runs.


def _gen_runs(ln):
    """Minimal cover of j=0..31 by 2-level affine runs of l, over period p."""
    best = None
    for p in range(1, 9):
        ops = []
        for r in range(p):
            js = list(range(r, _CROP, p))
            a = 0
            while a < len(js):
                b = a + 1
                P = None
                while b < len(js):
                    nP = int(ln[js[b]] - ln[js[b - 1]])
                    if P is None or nP == P:
                        P = nP
                        b += 1
                    else:
                        break
                ops.append(
                    (js[a], p, b - a, int(ln[js[a]]), P if P is not None else 0)
                )
                a = b
        if best is None or len(ops) < len(best):
            best = ops
    return best


def _plan(boxes, box_ind, n_emit=None):
    """Mirror the reference's float32 index math exactly. Returns
    (box plans, WLR table [128,16384] f16, V_packed [128, 32*nblk] f16)."""
    f32 = np.float32
    boxes = np.asarray(boxes, dtype=f32)
    N = boxes.shape[0]
    y1c, x1c, y2c, x2c = boxes[:, 0], boxes[:, 1], boxes[:, 2], boxes[:, 3]
    hs = (y2c - y1c) * f32(_H - 1) / f32(_CROP - 1)
    ws = (x2c - x1c) * f32(_W - 1) / f32(_CROP - 1)
    ii = np.arange(_CROP, dtype=f32)
    in_y = y1c[:, None] * f32(_H - 1) + ii[None, :] * hs[:, None]  # [N, 32]
    in_x = x1c[:, None] * f32(_W - 1) + ii[None, :] * ws[:, None]
    vy = (in_y >= 0) & (in_y <= _H - 1)
    vx = (in_x >= 0) & (in_x <= _W - 1)
    top_f = np.floor(in_y)
    left_f = np.floor(in_x)
    ly = (in_y - top_f).astype(f32)
    lx = (in_x - left_f).astype(f32)
    t = np.clip(top_f, 0, _H - 1).astype(np.int64)
    l = np.clip(left_f, 0, _W - 1).astype(np.int64)
    wt = np.where(vy, 1 - ly, 0).astype(f32)
    wb = np.where(vy, ly, 0).astype(f32)
    wl = np.where(vx, 1 - lx, 0).astype(f32)
    wr = np.where(vx, lx, 0).astype(f32)
    # Make the tap pair physically (l, l+1) / (t, t+1): at the top clip edge
    # the reference collapses to a single tap; shift down one with weight 0.
    ml = l == _W - 1
    wr = np.where(ml, wl + wr, wr)
    wl = np.where(ml, 0, wl)
    l = np.where(ml, _W - 2, l)
    mt = t == _H - 1
    wb = np.where(mt, wt + wb, wb)
    wt = np.where(mt, 0, wt)
    t = np.where(mt, _H - 2, t)

    n_emit = N if n_emit is None else n_emit

    plans = []
    for n in range(n_emit):
        bx = _Box()
        bx.n = n
        bx.b0 = int(box_ind[n])
        y0 = int(t[n].min())
        y1 = int(t[n].max()) + 1
        if y1 <= 127:
            # slabA, lanes p = y in [0, y1]
            seg_bounds = [("A", 0, y1 + 1, 0, y1, 0)]
        elif y0 >= 32:
            # slabB, lanes p = y-32 in [0, y1-32]
            seg_bounds = [("B", 0, y1 - 31, y0, y1, 32)]
        else:
            # split at 127/128: slabA rows y0..127 (p=y), slabB rows
            # 128..y1 (p=y-32 in [96, y1-32])
            seg_bounds = [
                ("A", 0, 128, 0, 127, 0),
                ("B", 96, y1 - 127, 128, y1, 32),
            ]
        bx.segs = seg_bounds  # vblk assigned after sorting
        bx.runs = [
            (
                j0,
                p,
                alen,
                l0,
                P,
                tuple(
                    (float(wl[n, j0 + p * a]), float(wr[n, j0 + p * a]))
                    for a in range(alen if alen <= 4 else 0)
                ),
            )
            for (j0, p, alen, l0, P) in _gen_runs(l[n])
        ]
        plans.append(bx)

    # emission order: pure-A boxes (grouped by batch, tracking slab-chunk
    # arrival) first, then B/split boxes (slabB is staged last)
    plans.sort(
        key=lambda bx: (
            0 if (bx.segs[0][0] == "A" and len(bx.segs) == 1) else 1,
            bx.b0,
            bx.n,
        )
    )

    # tables indexed by emission rank so chunked table DMAs gate only
    # later-ranked boxes
    wlr = np.zeros((n_emit * 64,), np.float32)
    vcols = []
    for rank, bx in enumerate(plans):
        bx.rank = rank
        wlr[rank * 64 + 0 : rank * 64 + 64 : 2] = wl[bx.n]
        wlr[rank * 64 + 1 : rank * 64 + 64 : 2] = wr[bx.n]
        segs = []
        for slab, pbase, K, ylo, yhi, pshift in bx.segs:
            blk = np.zeros((128, 32), np.float32)
            for i in range(_CROP):
                ti = int(t[bx.n, i])
                if ylo <= ti <= yhi:
                    blk[ti - pshift, i] += wt[bx.n, i]
                if ylo <= ti + 1 <= yhi:
                    blk[ti + 1 - pshift, i] += wb[bx.n, i]
            segs.append((slab, pbase, K, len(vcols)))
            vcols.append(blk)
        bx.segs = segs

    wlr_tile = np.broadcast_to(
        wlr.astype(np.float16), (128, n_emit * 64)
    ).copy()
    vpk = np.concatenate(vcols, axis=1).astype(np.float16)  # [128, 32*nblk]
    return plans, wlr_tile, vpk


# ------------------------------------------------------- device program ---
class _Bal:
    """Greedy DVE/GPS/ACT load balancer (costs ns, fit from NTFF traces)."""

    def __init__(self):
        self.load = {"V": 0.0, "G": 0.0, "A": 0.0}

    def pick(self, fd, act_ok=False, act_cost=None):
        cands = {"V": 1.55 * fd + 230.0, "G": 2.15 * fd + 440.0}
        if act_ok:
            cands["A"] = act_cost
        best = min(cands, key=lambda e: self.load[e] + cands[e])
        self.load[best] += cands[best]
        return best

    def charge(self, eng, ns):
        self.load[eng] += ns


def _build_program(plans, nblk):
    import concourse.bass as bass
    import concourse.mybir as mybir
    from concourse.ap import AP
    from concourse.tile import TileContext

    AL = mybir.AluOpType
    f16 = mybir.dt.float16
    fp32 = mybir.dt.float32
    n_emit = len(plans)
    nc = bass.Bass()
    imgA_p = nc.declare_dram_parameter("imgA", [128, 32768], f16, isOutput=False)
    imgC_p = nc.declare_dram_parameter("imgC", [32, 32768], f16, isOutput=False)
    wlr_p = nc.declare_dram_parameter(
        "wlr", [128, n_emit * 64], f16, isOutput=False
    )
    vpk_p = nc.declare_dram_parameter("vpk", [128, 32 * nblk], f16, isOutput=False)
    out_p = nc.declare_dram_parameter(
        "out", [n_emit, 32, 1024], f16, isOutput=True
    )
    bal = _Bal()
    with TileContext(nc) as tc:
        with (
            tc.tile_pool(name="img", bufs=1) as imgp,
            tc.tile_pool(name="prod", bufs=4) as prodp,
            tc.tile_pool(name="psum", bufs=4, space="PSUM") as psump,
            tc.tile_pool(name="outp", bufs=6) as outp,
        ):
            SLA = imgp.tile([128, 32768], f16)
            SLB = imgp.tile([128, 32768], f16)
            WLR = imgp.tile([128, n_emit * 64], f16)
            VPK = imgp.tile([128, 32 * nblk], f16)
            # Staging is interleaved into the box loop (emitted just before
            # the first box of each batch group) so early boxes' output DMAs
            # aren't queued behind all 28MB of staging on the DMA engines.
            # plans are sorted (pure-A by batch, then B/split), and tables
            # are rank-indexed, so each group's table columns are a
            # contiguous range.
            groups = {}
            for r, bx in enumerate(plans):
                gk = (
                    0 if (bx.segs[0][0] == "A" and len(bx.segs) == 1) else 1,
                    bx.b0 if (bx.segs[0][0] == "A" and len(bx.segs) == 1) else -1,
                )
                lo_r, hi_r, lo_v, hi_v = groups.get(
                    gk, (r, r, bx.segs[0][3], bx.segs[-1][3])
                )
                groups[gk] = (
                    min(lo_r, r),
                    max(hi_r, r),
                    min(lo_v, bx.segs[0][3]),
                    max(hi_v, bx.segs[-1][3]),
                )

            staged = set()

            def stage_group(gk):
                if gk in staged or gk not in groups:
                    return
                staged.add(gk)
                lo_r, hi_r, lo_v, hi_v = groups[gk]
                ws = slice(lo_r * 64, (hi_r + 1) * 64)
                vs = slice(lo_v * 32, (hi_v + 1) * 32)
                nc.sync.dma_start(out=WLR[:, ws], in_=wlr_p[:, ws])
                nc.sync.dma_start(out=VPK[:, vs], in_=vpk_p[:, vs])
                if gk[0] == 0:
                    cs = slice(gk[1] * 8192, (gk[1] + 1) * 8192)
                    nc.sync.dma_start(out=SLA[:, cs], in_=imgA_p[:, cs])
                    if gk[1] == _B - 1:
                        # slabB fill once all slabA chunks are staged; B/split
                        # boxes run at the end, well after this drains
                        for b in range(_B):
                            cb = slice(b * 8192, (b + 1) * 8192)
                            nc.sync.dma_start(
                                out=SLB[0:96, cb], in_=SLA[32:128, cb]
                            )
                            nc.sync.dma_start(
                                out=SLB[96:128, cb], in_=imgC_p[:, cb]
                            )
                else:
                    # B/split group: ensure every slab chunk is staged
                    for b in range(_B):
                        stage_group((0, b))
            h_sla = SLA[0:1, 0:1].tensor
            h_slb = SLB[0:1, 0:1].tensor
            h_wlr = WLR[0:1, 0:1].tensor

            # quad-batched psum drain: 4 boxes share one [128,1024] psum via
            # PE column groups; one full-width ACT copy drains all four
            quad = []
            psum128 = None
            outst128 = None

            def flush_quad():
                nonlocal quad, psum128, outst128
                if not quad:
                    return
                P = 32 * len(quad)
                nc.scalar.mul(
                    out=outst128[0:P, :], in_=psum128[0:P, :], mul=1.0
                )
                bal.charge("A", 300.0 + 0.8 * P * 8.0)
                for qi, qn in enumerate(quad):
                    nc.sync.dma_start(
                        out=out_p[qn], in_=outst128[32 * qi : 32 * qi + 32, :]
                    )
                quad = []

            for bx in plans:
                n = bx.n
                rank = bx.rank
                b0 = bx.b0
                stage_group(
                    (
                        0
                        if (bx.segs[0][0] == "A" and len(bx.segs) == 1)
                        else 1,
                        b0
                        if (bx.segs[0][0] == "A" and len(bx.segs) == 1)
                        else -1,
                    )
                )
                q = len(quad)
                if q == 0:
                    psum128 = psump.tile([128, 1024], fp32)
                    outst128 = outp.tile([128, 1024], f16, tag="outst")
                quad.append(n)
                psum = psum128[32 * q : 32 * (q + 1), :]
                nseg = len(bx.segs)
                for s, (slab, pbase, K, vblk) in enumerate(bx.segs):
                    h_sl = h_sla if slab == "A" else h_slb
                    prod = prodp.tile([128, 2048], f16, tag="prod")
                    h_prod = prod[0:1, 0:1].tensor
                    wlr_fs = n_emit * 64
                    for (j0, p, alen, l0, P, ws) in bx.runs:
                        fd = 64 * alen
                        eng = bal.pick(
                            fd,
                            act_ok=(alen <= 4),
                            act_cost=alen * 2 * 310.0,
                        )
                        if eng == "A":
                            # tiny run: immediate-scalar muls on ACT
                            for a, (wl0, wr0) in enumerate(ws):
                                sbase = (
                                    pbase * 32768
                                    + b0 * 8192
                                    + (l0 + P * a) * 32
                                )
                                obase = pbase * 2048 + (j0 + p * a) * 32
                                for tt, w in ((0, wl0), (1, wr0)):
                                    i0 = AP(
                                        h_sl,
                                        sbase + 32 * tt,
                                        [[32768, K], [1, 32]],
                                    )
                                    ot = AP(
                                        h_prod,
                                        obase + 1024 * tt,
                                        [[2048, K], [1, 32]],
                                    )
                                    nc.scalar.mul(out=ot, in_=i0, mul=w)
                            continue
                        in0 = AP(
                            h_sl,
                            pbase * 32768 + b0 * 8192 + l0 * 32,
                            [[32768, K], [32 * P, alen], [32, 2], [1, 32]],
                        )
                        in1 = AP(
                            h_wlr,
                            pbase * wlr_fs + rank * 64 + j0 * 2,
                            [[wlr_fs, K], [2 * p, alen], [1, 2], [0, 32]],
                        )
                        o = AP(
                            h_prod,
                            pbase * 2048 + j0 * 32,
                            [[2048, K], [32 * p, alen], [1024, 2], [1, 32]],
                        )
                        if eng == "V":
                            nc.vector.tensor_tensor(o, in0, in1, AL.mult)
                        else:
                            nc.gpsimd.tensor_tensor(o, in0, in1, AL.mult)
                    # vertical lerp + tap fold on PE: psum[i, (j,c)] =
                    # sum_t V_seg^T @ prod[:, t-half]; all rhs contiguous.
                    tp = (pbase, 32 * q)
                    lhsT = VPK[pbase : pbase + K, vblk * 32 : (vblk + 1) * 32]
                    for t in (0, 1):
                        for h in (0, 1):
                            rhs = prod[
                                pbase : pbase + K,
                                t * 1024 + h * 512 : t * 1024 + (h + 1) * 512,
                            ]
                            nc.tensor.matmul(
                                psum[:, h * 512 : (h + 1) * 512],
                                lhsT,
                                rhs,
                                start=(s == 0 and t == 0),
                                stop=(s == nseg - 1 and t == 1),
                                tile_position=tp,
                            )
                if len(quad) == 4:
                    flush_quad()
            flush_quad()
    _split_multi_waits(nc)
    return nc


# ------------------------------------------------------------- execution ---
def _make_spmd_exec(nc, n_cores):
    """Persistent shard_map'd jitted callable running the SAME bass program
    on n_cores devices with device-resident args."""
    import jax
    import concourse.mybir as mybir
    from concourse import bass2jax
    from jax.experimental.shard_map import shard_map
    from jax.sharding import Mesh, NamedSharding, PartitionSpec

    bass2jax.install_neuronx_cc_hook()

    partition_name = (
        nc.partition_id_tensor.name if nc.partition_id_tensor else None
    )
    in_names, out_names, out_avals, zero_outs = [], [], [], []
    for alloc in nc.m.functions[0].allocations:
        if not isinstance(alloc, mybir.MemoryLocationSet):
            continue
        name = alloc.memorylocations[0].name
        if alloc.kind == "ExternalInput":
            if name != partition_name:
                in_names.append(name)
        elif alloc.kind == "ExternalOutput":
            out_names.append(name)
            shape = tuple(alloc.tensor_shape)
            dtype = mybir.dt.np(alloc.dtype)
            out_avals.append(jax.core.ShapedArray(shape, dtype))
            zero_outs.append(np.zeros(shape, dtype))
    n_params = len(in_names)
    n_outs = len(out_avals)
    all_names = in_names + out_names
    if partition_name is not None:
        all_names = all_names + [partition_name]

    def _body(*args):
        operands = list(args)
        if partition_name is not None:
            operands.append(bass2jax.partition_id_tensor())
        outs = bass2jax._bass_exec_p.bind(
            *operands,
            out_avals=tuple(out_avals),
            in_names=tuple(all_names),
            out_names=tuple(out_names),
            lowering_input_output_aliases=(),
            sim_require_finite=True,
            sim_require_nnan=True,
            nc=nc,
        )
        return tuple(outs)

    devices = jax.devices()[:n_cores]
    assert len(devices) == n_cores, devices
    mesh = Mesh(np.asarray(devices), ("core",))
    sharded = jax.jit(
        shard_map(
            _body,
            mesh=mesh,
            in_specs=(PartitionSpec("core"),) * (n_params + n_outs),
            out_specs=(PartitionSpec("core"),) * len(out_names),
            check_rep=False,
        ),
        keep_unused=True,
    )
    sharding = NamedSharding(mesh, PartitionSpec("core"))
    return sharded, in_names[:n_params], out_names, zero_outs, sharding


class Runner:
    def __init__(self, image, boxes, box_ind, n_emit=None):
        import jax

        _apply_bass_patches()
        image = np.ascontiguousarray(np.asarray(image, dtype=np.float32))
        boxes = np.asarray(boxes, dtype=np.float32)
        box_ind = np.asarray(box_ind, dtype=np.int32)
        self.n_boxes = boxes.shape[0]

        plans, wlr_tile, vpk = _plan(boxes, box_ind, n_emit)
        self.n_emit = len(plans)
        nblk = vpk.shape[1] // 32
        nc = _build_program(plans, nblk)
        self.nc = nc
        sharded, in_names, out_names, zero_outs, sharding = _make_spmd_exec(
            nc, _NCORE
        )
        self.sharded = sharded
        self.out_names = out_names

        # per-core slabs: core k holds channels [32k, 32k+32).
        img16 = image.astype(np.float16)  # [B, C, H, W]
        # slab col layout: b*8192 + x*32 + c  (c innermost)
        imgA = np.stack(
            [
                img16[:, k * _CPC : (k + 1) * _CPC, 0:128]
                .transpose(2, 0, 3, 1)
                .reshape(128, 32768)
                for k in range(_NCORE)
            ]
        )  # [8, 128, 32768]
        imgC = np.stack(
            [
                img16[:, k * _CPC : (k + 1) * _CPC, 128:160]
                .transpose(2, 0, 3, 1)
                .reshape(32, 32768)
                for k in range(_NCORE)
            ]
        )
        in_map = {
            "imgA": imgA.reshape(_NCORE * 128, 32768),
            "imgC": imgC.reshape(_NCORE * 32, 32768),
            "wlr": np.tile(wlr_tile, (_NCORE, 1)),
            "vpk": np.tile(vpk, (_NCORE, 1)),
        }
        args = [jax.device_put(in_map[n], sharding) for n in in_names]
        args += [
            jax.device_put(
                np.zeros((_NCORE * z.shape[0], *z.shape[1:]), z.dtype), sharding
            )
            for z in zero_outs
        ]
        jax.block_until_ready(args)
        self.args = args

    def run(self):
        import jax

        outs = self.sharded(*self.args)
        jax.block_until_ready(outs)
        return outs

    def gather(self, outs):
        ne = self.n_emit
        res = {name: o for name, o in zip(self.out_names, outs)}
        arr = (
            np.asarray(res["out"])
            .astype(np.float32)
            .reshape(_NCORE, ne, 32, 32, 32)  # [k, n, i, j, c]
        )
        out = np.ascontiguousarray(
            arr.transpose(1, 0, 4, 2, 3).reshape(ne, _C, _CROP, _CROP)
        )
        return out


def kernel(image, boxes, box_ind):
    r = Runner(image, boxes, box_ind)
    return r.gather(r.run())
